# revision 51
# baseline (speedup 1.0000x reference)
"""Trainium2 Bass kernel for nn_AttentionApproximator (sparse_attention).

Math (per batch b):
  scores = relu(full @ sw1 + sb1) @ sw2 (+ sb2, rank-irrelevant)   [S]
  top_idx = top_k(scores, k=204)                                    (set only)
  sel     = full[top_idx]                                           [k, d]
  q_part  = full @ mw1[:d]                                          [S, 64]
  kvb     = sel @ (mw1[d:2d] + mw1[2d:]) + mb1                      [k, 64]
  h1      = relu(q_part[s] + kvb[j])                                [S, k, 64]
  h2      = relu(h1 @ mw2 + mb2)                                    [S, k, 32]
  out     = mean_j(h2) @ mw3 + mb3                                  [S, d]
        (the mw3 matmul commutes with the j-mean)

Device strategy (8 cores, SPMD): core c handles batch b=c//2, query rows
h=c%2 (1024 of 2048).  Top-k via exact ranks (rank_i = #{j: s_j > s_i});
rank doubles as the compaction slot, gather is a one-hot matmul.

Perf structure vs the reference implementation:
  - scores/bc matmuls run as float32r (1 cycle/row) via bitcast; the
    broadcast bc[p,j]=s_j comes directly out of a tiled-sw2 matmul, so both
    score layouts derive from one tensor (comparison consistency).
  - rank chunks are split across DVE (is_gt+accum) and ACT (Sign+accum);
    one-hots on GPSIMD (trailing ones on DVE); kv copies ride the rank
    window split DVE/ACT (GPSIMD cannot touch PSUM).
  - main loop: h1 = relu(qT2 + kvb) on DVE in 4x perf mode (all-bf16,
    all-SBUF); mw2 in bf16; h2 relu written as fp8e4 into per-supergroup
    [128, 2*1024] tiles (split ACT/DVE; some h1 pairs on GPSIMD); the mw3
    contraction runs as fp8 DoubleRow matmuls covering 8 tokens each
    (0.5 cyc/row), with per-copy dither quantization of mw3.
  - w4 emission is delayed one supergroup so the in-order PE queue never
    stalls waiting for the h2 relu of the newest group.
"""

import os
from contextlib import ExitStack

import numpy as np

B, S, D = 4, 2048, 16
DA = D + 1               # augmented with ones row
K = 204                  # top-k  (int(2048*0.1))
KP = K // 2              # 102 slot-pairs
H1 = 64
H2 = 32
SH = S // 2              # 1024 query rows per core
NCH = S // 128           # 16 token chunks
N_CORES = 8
NG = KP // 2             # 51 groups of 4 tokens
MW3_SCALE = 16.0         # host-side mw3 prescale (exact power of 2)
INV_K = float(np.float32(1.0) / np.float32(K)) / MW3_SCALE

# engine split knobs
RANK_D = int(os.environ.get("KERNEL_RANK_D", "11"))  # DVE rank chunks
RANK_A = int(os.environ.get("KERNEL_RANK_A", "5"))   # ACT rank chunks (rest
# would go to GPSIMD, but walrus rejects accum_out on Pool -- keep D+A=16)
H2_SPLIT = int(os.environ.get("KERNEL_H2_SPLIT", "26"))  # h2 groups split ACT+DVE
H1_POOL = int(os.environ.get("KERNEL_H1_POOL", "16"))  # h1 pairs on GPSIMD
# DoubleRow mw2 is dead on TRN2: the ISA requires DR matmul dst base
# partition 0 (s3d3_mm_valid_dst_partition), so the second token-pair
# (rows 64:128) cannot be produced by a DR matmul. Keep 0.
N_DR = int(os.environ.get("KERNEL_N_DR", "0"))
RANK_D = min(RANK_D, NCH)
RANK_A = NCH - RANK_D

_cache = {}


def _build_module():
    import concourse.mybir as mybir
    import concourse.tile as tile
    from concourse import bacc

    fp32 = mybir.dt.float32
    f32r = mybir.dt.float32r
    bf16 = mybir.dt.bfloat16
    fp8 = mybir.dt.float8e4
    Alu = mybir.AluOpType
    Act = mybir.ActivationFunctionType
    DR = mybir.MatmulPerfMode.DoubleRow

    nc = bacc.Bacc("TRN2", target_bir_lowering=False, debug=False,
                   num_devices=N_CORES)

    # ---- DRAM I/O (order below = DMA issue order; HWDGE is serial) ----
    # pksc: [0:17,0:32]=sw1a | [0:32,32:160]=sw2 tiled 128x | [17,0]=1.0
    d_pksc = nc.dram_tensor("pksc", [32, 160], f32r, kind="ExternalInput").ap()
    d_fbT = nc.dram_tensor("fbT", [DA, S], f32r, kind="ExternalInput").ap()
    d_fqT = nc.dram_tensor("fqT", [D, SH], bf16, kind="ExternalInput").ap()
    d_fbTb = nc.dram_tensor("fbTb", [DA, S], bf16, kind="ExternalInput").ap()
    # pkf: cols 0:204 iota (even slots 0:102, odd 102:204) | 204 mb24 | 205 mb3f
    d_pkf = nc.dram_tensor("pkf", [128, 208], fp32, kind="ExternalInput").ap()
    # pkb: cols 0:64 bdmw2 | [0:16,64:128] wq | [0:17,128:192] wkv
    d_pkb = nc.dram_tensor("pkb", [128, 192], bf16, kind="ExternalInput").ap()
    # pk8: 8 dither-quantized copies of mw3*16 laid out as [128 = 4 rank
    # blocks x 32, 2 DR k-tiles, 16]; per-weight the copies alternate
    # between the two nearest fp8 values so the tiled copies average out
    # the systematic quantization error
    d_pk8 = nc.dram_tensor("pk8", [128, 96], fp8, kind="ExternalInput").ap()

    d_outT = nc.dram_tensor("outT", [D, SH], fp32, kind="ExternalOutput").ap()

    # round-robin the rank chunks through the D/A/P multiset
    rank_asgn = []
    pool_left = {"D": RANK_D, "A": RANK_A, "P": NCH - RANK_D - RANK_A}
    cyc = ["D", "A", "P"]
    ci = 0
    for c in range(NCH):
        for _ in range(3):
            e = cyc[ci % 3]
            ci += 1
            if pool_left[e] > 0:
                pool_left[e] -= 1
                rank_asgn.append(e)
                break
        else:
            rank_asgn.append("D")

    # h2 engine per group: "S" = split halves across ACT+DVE, else ACT
    # (GPSIMD cannot read PSUM so h2 is ACT/DVE only)
    h2_asgn = []
    accS = 0.0
    for g in range(NG):
        accS += H2_SPLIT / NG
        if accS >= 1.0:
            accS -= 1.0
            h2_asgn.append("S")
        else:
            h2_asgn.append("A")
    # DR groups (fp8 DoubleRow mw2): spread evenly, never the tail group
    dr_asgn = [False] * NG
    accR = 0.0
    for g in range(NG - 1):
        accR += N_DR / (NG - 1)
        if accR >= 1.0:
            accR -= 1.0
            dr_asgn[g] = True
    # h1 pair engine: GPSIMD takes fp8 (DR-group) pairs first, then bf16
    # pairs, up to H1_POOL; the rest on DVE
    fp8_pairs = [p for p in range(KP) if dr_asgn[p // 2]]
    bf_pairs = [p for p in range(KP) if not dr_asgn[p // 2]]
    pool_set = set((fp8_pairs + bf_pairs)[:H1_POOL])
    h1_asgn = ["P" if p in pool_set else "D" for p in range(KP)]

    with tile.TileContext(nc) as tc:
        with (
            ExitStack() as es,
            tc.tile_pool(name="const", bufs=1) as cpool,
            tc.tile_pool(name="sel", bufs=1) as spool,
            tc.tile_pool(name="scrD", bufs=2) as scrDp,
            tc.tile_pool(name="scrA", bufs=2) as scrAp,
            tc.tile_pool(name="scrP", bufs=2) as scrPp,
            tc.tile_pool(name="h1p", bufs=4) as h1pool,
            tc.tile_pool(name="h2p", bufs=3) as h2pool,
        ):
            # ---- input DMAs (critical ones first) ----
            sb_pksc = cpool.tile([32, 160], f32r)
            nc.sync.dma_start(sb_pksc[:], d_pksc[:])
            sb_fbT = cpool.tile([DA, S], f32r)
            nc.sync.dma_start(sb_fbT[:], d_fbT[:])
            sb_fqT = cpool.tile([D, SH], bf16)
            nc.sync.dma_start(sb_fqT[:], d_fqT[:])
            sb_fbTb = cpool.tile([DA, S], bf16)
            nc.sync.dma_start(sb_fbTb[:], d_fbTb[:])
            sb_pkf = cpool.tile([128, 208], fp32)
            nc.sync.dma_start(sb_pkf[:], d_pkf[:])
            sb_pkb = cpool.tile([128, 192], bf16)
            nc.sync.dma_start(sb_pkb[:], d_pkb[:])
            sb_pk8 = cpool.tile([128, 96], fp8)
            nc.sync.dma_start(sb_pk8[:], d_pk8[:])

            sb_sw1a = sb_pksc[0:DA, 0:32]
            sb_sw2t = sb_pksc[0:H2, 32:160]
            sb_one = sb_pkf[0:1, 206:207]
            sb_iota = sb_pkf[:, 0:K]
            sb_mb24 = sb_pkf[:, 204:205]
            sb_mb3f = sb_pkf[0:D, 205:206]
            sb_bdmw2 = sb_pkb[:, 0:H1]
            sb_wq = sb_pkb[0:D, 64:128]
            sb_wkv = sb_pkb[0:DA, 128:192]

            # ---- PE warmup while input DMAs stream ----
            with tc.tile_pool(name="pswarm", bufs=1, space="PSUM") as pW:
                ps_w = pW.tile([32, 160], fp32)
                nc.tensor.matmul(ps_w[:], sb_pksc[0:32, 0:32],
                                 sb_pksc[0:32, :], start=True, stop=True)

            # ---- stage A: scores hidden layer + bc broadcast ----
            # bc[p, j] = s_j for every partition p, via sw2-tiled lhsT; the
            # single bc tensor is the source of truth for all score compares.
            sb_bc = spool.tile([128, S], fp32)
            sb_Hs = []
            with tc.tile_pool(name="psA", bufs=4, space="PSUM") as pA, \
                 tc.tile_pool(name="psBC", bufs=4, space="PSUM") as pBC:
                for n in range(4):
                    sl = slice(n * 512, (n + 1) * 512)
                    ps_Hn = pA.tile([H2, 512], fp32, tag="H")
                    nc.tensor.matmul(ps_Hn[:], sb_sw1a, sb_fbT[:, sl],
                                     start=True, stop=True)
                    sb_Hn = spool.tile([H2, 512], f32r, tag=f"sbH{n}")
                    if n % 2 == 0:
                        nc.scalar.activation(sb_Hn[:], ps_Hn[:], Act.Relu)
                    else:
                        nc.vector.tensor_scalar(sb_Hn[:], ps_Hn[:], 0.0, None,
                                                Alu.max)
                    sb_Hs.append(sb_Hn)
                    ps_bcn = pBC.tile([128, 512], fp32, tag="bc")
                    nc.tensor.matmul(ps_bcn[:], sb_sw2t, sb_Hn[:],
                                     start=True, stop=True)
                    if n % 2 == 1:
                        nc.vector.tensor_copy(sb_bc[:, sl], ps_bcn[:])
                    else:
                        nc.scalar.copy(sb_bc[:, sl], ps_bcn[:])

            # ---- stage A2: token-major scores via PE transposes ----
            with tc.tile_pool(name="psA2", bufs=1, space="PSUM") as pA2:
                ps_stok = pA2.tile([128, NCH], fp32)
                for c in range(NCH):
                    nc.tensor.transpose(
                        ps_stok[:, c:c + 1],
                        sb_bc[0:1, c * 128:(c + 1) * 128], sb_one)
                sb_stok = spool.tile([128, NCH], fp32)
                nc.vector.tensor_copy(sb_stok[:], ps_stok[:])


            # ---- stage C1: kv for all tokens (bf16; GPSIMD copies) ----
            es_c = ExitStack()
            pC = es_c.enter_context(tc.tile_pool(name="psC", bufs=2,
                                                 space="PSUM"))
            pCs = es_c.enter_context(tc.tile_pool(name="psCsel", bufs=1,
                                                  space="PSUM"))
            sb_kvtoks = []
            kv_copy_jobs = []
            for c in range(NCH):
                ps_kv = pC.tile([128, H1], fp32, tag="kvtok")
                nc.tensor.matmul(ps_kv[:],
                                 sb_fbTb[:, c * 128:(c + 1) * 128],
                                 sb_wkv, start=True, stop=True)
                sb_kvc = spool.tile([128, H1], bf16, tag=f"kvtok{c}")
                kv_copy_jobs.append((sb_kvc, ps_kv))
                sb_kvtoks.append(sb_kvc)

            # ---- stage D: query part stacked twice [128, SH] bf16 ----
            # (matmuls now; the ACT copy is emitted after ACT's rank chunks)
            es_d = ExitStack()
            pD = es_d.enter_context(tc.tile_pool(name="psD", bufs=1,
                                                 space="PSUM"))
            ps_q = pD.tile([128, SH], fp32)
            for half in range(2):
                rows = slice(half * H1, (half + 1) * H1)
                for n in range(2):
                    sl = slice(n * 512, (n + 1) * 512)
                    nc.tensor.matmul(ps_q[rows, sl], sb_wq,
                                     sb_fqT[:, sl], start=True, stop=True)
            sb_qT2 = cpool.tile([128, SH], bf16)

            # ---- stage B: exact ranks, 3-engine split; kv copies ride in
            # the DVE stream between rank chunks (no rank dependency) ----
            sb_ranks = []
            for c in range(NCH):
                rk = spool.tile([128, 1], fp32, tag=f"rank{c}")
                sb_ranks.append(rk)
            kv_iter = iter(kv_copy_jobs)
            kv_per_d = (len(kv_copy_jobs) + RANK_D - 1) // max(RANK_D, 1)
            for c in range(NCH):
                e = rank_asgn[c]
                if e == "A":
                    scr = scrAp.tile([128, S], fp32, tag="scrA")
                    rsgn = spool.tile([128, 1], fp32, tag=f"rsgn{c}")
                    nc.scalar.activation(scr[:], sb_bc[:], Act.Sign,
                                         bias=sb_stok[:, c:c + 1],
                                         scale=-1.0, accum_out=rsgn[:])
                    nc.vector.tensor_scalar(sb_ranks[c][:], rsgn[:], -0.5,
                                            1023.5, Alu.mult, Alu.add)
                elif e == "D":
                    scr = scrDp.tile([128, S], fp32, tag="scrD")
                    nc.vector.tensor_scalar(
                        scr[:], sb_bc[:], sb_stok[:, c:c + 1], 0.0,
                        Alu.is_gt, Alu.add, accum_out=sb_ranks[c][:])
                    for _ in range(kv_per_d):
                        job = next(kv_iter, None)
                        if job is not None:
                            nc.vector.tensor_copy(job[0][:], job[1][:])
                else:
                    scr = scrPp.tile([128, S], fp32, tag="scrP")
                    nc.gpsimd.tensor_scalar(
                        scr[:], sb_bc[:], sb_stok[:, c:c + 1], 0.0,
                        Alu.is_gt, Alu.add, accum_out=sb_ranks[c][:])
            for job in kv_iter:
                nc.vector.tensor_copy(job[0][:], job[1][:])
            # qT2 copy rides after ACT's rank chunks
            nc.scalar.copy(sb_qT2[:], ps_q[:])
            es_d.close()

            # ---- stage C2: one-hot gather of the top-K kv rows (bf16) ----
            ps_kvsel_full = pCs.tile([128, 512], fp32)   # 1 bank
            ps_kvsel = ps_kvsel_full[:, 0:KP]
            for c in range(NCH):
                oh = scrPp.tile([128, K], bf16, tag="oh")
                nc.gpsimd.tensor_scalar(oh[:], sb_iota, sb_ranks[c][:], None,
                                        Alu.is_equal)
                kvch = sb_kvtoks[c][:]
                nc.tensor.matmul(ps_kvsel[0:H1, :], kvch, oh[:, 0:KP],
                                 start=(c == 0), stop=False,
                                 skip_group_check=True)
                nc.tensor.matmul(ps_kvsel[H1:128, :], kvch, oh[:, KP:K],
                                 start=(c == 0), stop=(c == NCH - 1),
                                 skip_group_check=True)
            sb_kvb2a = spool.tile([128, 51], fp32)
            nc.vector.tensor_copy(sb_kvb2a[:], ps_kvsel[:, 0:51])
            sb_kvb2b = spool.tile([128, KP - 51], fp32)
            nc.vector.tensor_copy(sb_kvb2b[:], ps_kvsel[:, 51:KP])
            es_c.close()

            def kvb_col(p):
                return (sb_kvb2a[:, p:p + 1] if p < 51
                        else sb_kvb2b[:, p - 51:p - 50])

            pmain = es.enter_context(
                tc.tile_pool(name="main_psum", bufs=3, space="PSUM"))
            pout = es.enter_context(
                tc.tile_pool(name="out_psum", bufs=1, space="PSUM"))

            # ---- stage E: main pairwise loop ----
            # Group g covers tokens 4g..4g+3 (2 slot-pairs).  h2 of two
            # consecutive groups shares one [128, 2048] fp8 supertile whose
            # halves are the DoubleRow k-tiles of the mw3 contraction.
            ps_out = pout.tile([D, SH], fp32, tag="outacc")
            lhsT_dr = sb_pk8[:, 0:32].rearrange("p (k m) -> p k m", k=2)
            sup = None
            pending_w4 = []
            first_w4 = [True]
            g_order = list(range(NG))

            def emit_w4(sup_t, last=False):
                rhs3 = sup_t[:, 0:2 * SH].rearrange("p (k q) -> p k q", k=2)
                for half in range(2):
                    sl = slice(half * 512, (half + 1) * 512)
                    nc.tensor.matmul(ps_out[:, sl], lhsT_dr, rhs3[:, :, sl],
                                     perf_mode=DR, start=first_w4[0],
                                     stop=last and half == 1,
                                     skip_group_check=True)
                first_w4[0] = False

            nreg = [0]
            for gi, g in enumerate(g_order):
                par = nreg[0] % 2
                if g != NG - 1:
                    nreg[0] += 1
                    if par == 0:
                        sup = h2pool.tile([128, 2 * SH], fp8, tag="sup")
                ps_h = pmain.tile([128, SH], fp32, tag="hps")
                for half in range(2):
                    p = 2 * g + half
                    h1t = h1pool.tile([128, SH], bf16, tag="h1")
                    if h1_asgn[p] == "P":
                        nc.gpsimd.tensor_scalar(h1t[:], sb_qT2[:],
                                                kvb_col(p), 0.0,
                                                Alu.add, Alu.max)
                    else:
                        nc.vector.tensor_scalar(h1t[:], sb_qT2[:],
                                                kvb_col(p), 0.0,
                                                Alu.add, Alu.max)
                    rows = slice(half * H1, (half + 1) * H1)
                    for n in range(2):
                        sl = slice(n * 512, (n + 1) * 512)
                        nc.tensor.matmul(ps_h[rows, sl], sb_bdmw2,
                                         h1t[:, sl], start=True, stop=True)
                if g == NG - 1:
                    # odd 51st group: fp8 tail
                    tail = h2pool.tile([128, SH], fp8, tag="tail")
                    h2dst = tail[:]
                    split = False
                else:
                    h2dst = sup[:, par * SH:(par + 1) * SH]
                    split = h2_asgn[g] != "A"
                if not split:
                    nc.scalar.activation(h2dst, ps_h[:], Act.Relu,
                                         bias=sb_mb24)
                else:
                    nc.scalar.activation(h2dst[:, 0:512], ps_h[:, 0:512],
                                         Act.Relu, bias=sb_mb24)
                    nc.vector.tensor_scalar(h2dst[:, 512:SH],
                                            ps_h[:, 512:SH], sb_mb24, 0.0,
                                            Alu.add, Alu.max)
                if g != NG - 1 and par == 1:
                    pending_w4.append(sup)
                    if len(pending_w4) > 1:
                        emit_w4(pending_w4.pop(0))
            for s_t in pending_w4:
                emit_w4(s_t)
            # tail group: two zero-ktile DR matmuls — the rhs k-tiles are the
            # two query-halves of the tail tile; the unused half multiplies
            # zero weights (pk8 cols 32:96 hold [w3|0] and [0|w3])
            tail3 = tail[:, 0:SH].rearrange("p (k q) -> p k q", k=2)
            lhsT_t0 = sb_pk8[:, 32:64].rearrange("p (k m) -> p k m", k=2)
            lhsT_t1 = sb_pk8[:, 64:96].rearrange("p (k m) -> p k m", k=2)
            nc.tensor.matmul(ps_out[:, 0:512], lhsT_t0, tail3,
                             perf_mode=DR, start=False, stop=False,
                             skip_group_check=True)
            nc.tensor.matmul(ps_out[:, 512:SH], lhsT_t1, tail3,
                             perf_mode=DR, start=False, stop=True,
                             skip_group_check=True)

            # ---- stage F: scale + bias + store ----
            sb_out = spool.tile([D, SH], fp32)
            nc.scalar.activation(sb_out[:], ps_out[:], Act.Identity,
                                 bias=sb_mb3f, scale=INV_K)
            nc.sync.dma_start(d_outT[:], sb_out[:])

    nc.compile()
    return nc


def _host_inputs(full, sw1, sb1, sw2, sb2, mw1, mb1, mw2, mb2, mw3, mb3):
    """Build the 8 per-core input maps (host-side sharding + layout prep)."""
    import ml_dtypes
    f32 = np.float32
    bf16 = ml_dtypes.bfloat16
    fp8 = ml_dtypes.float8_e4m3
    full = np.asarray(full, dtype=f32)
    ones_row = np.ones((1, S), dtype=f32)

    pksc = np.zeros((32, 160), dtype=f32)
    pksc[0:DA, 0:32] = np.concatenate(
        [np.asarray(sw1, dtype=f32), np.asarray(sb1, dtype=f32)[None, :]],
        axis=0)
    pksc[0:H2, 32:160] = np.tile(np.asarray(sw2, dtype=f32).reshape(H2, 1),
                                 (1, 128))

    pkf = np.zeros((128, 208), dtype=f32)
    pkf[0, 206] = 1.0
    pkf[:, 0:KP] = np.arange(0, K, 2, dtype=f32)[None, :]
    pkf[:, KP:K] = np.arange(1, K, 2, dtype=f32)[None, :]
    pkf[:, 204] = np.tile(np.asarray(mb2, dtype=f32), 4)
    pkf[:, 205] = np.tile(np.asarray(mb3, dtype=f32), 8)

    pkb = np.zeros((128, 192), dtype=f32)
    pkb[0:H1, 0:H2] = mw2
    pkb[H1:128, H2:H1] = mw2
    pkb[0:D, 64:128] = mw1[:D]
    pkb[0:DA, 128:192] = np.concatenate(
        [np.asarray(mw1[D:2 * D] + mw1[2 * D:], dtype=f32),
         np.asarray(mb1, dtype=f32)[None, :]], axis=0)
    pkb = pkb.astype(bf16)

    # 8 dither-quantized copies of mw3*16: copy (block b, ktile i) at
    # pk8[32b:32b+32, 16i:16i+16]; per weight, copies alternate between
    # the two fp8 neighbours bracketing the true value so that the
    # average over copies tracks it to ~1/16 ulp
    W = np.asarray(mw3, dtype=f32) * MW3_SCALE            # [32, 16]
    fp8_vals = np.sort(np.unique(
        np.arange(256, dtype=np.uint8).view(fp8).astype(f32)))
    fp8_vals = fp8_vals[np.isfinite(fp8_vals)]
    lo_idx = np.searchsorted(fp8_vals, W, side="right") - 1
    lo = fp8_vals[np.clip(lo_idx, 0, len(fp8_vals) - 1)]
    hi = fp8_vals[np.clip(lo_idx + 1, 0, len(fp8_vals) - 1)]
    denom = np.where(hi > lo, hi - lo, 1.0)
    n_hi = np.round((W - lo) / denom * 8.0)
    pk8 = np.zeros((128, 96), dtype=f32)
    for b in range(4):
        for i in range(2):
            c = 2 * b + i
            cpy = np.where(c < n_hi, hi, lo)
            pk8[32 * b:32 * b + 32, 16 * i:16 * i + 16] = cpy
            # tail lhsTs: [w3|0] at cols 32:64, [0|w3] at 64:96
            pk8[32 * b:32 * b + 32, 32 + 16 * i:48 + 16 * i] = (
                cpy if i == 0 else 0.0)
            pk8[32 * b:32 * b + 32, 64 + 16 * i:80 + 16 * i] = (
                0.0 if i == 0 else cpy)
    pk8 = pk8.astype(fp8)

    shared = dict(pksc=pksc, pkf=pkf, pkb=pkb, pk8=pk8)
    in_maps = []
    for c in range(N_CORES):
        b, h = c // 2, c % 2
        fbT = np.concatenate(
            [np.ascontiguousarray(full[b].T), ones_row], axis=0)
        m = dict(shared)
        m["fbT"] = fbT
        m["fbTb"] = fbT.astype(bf16)
        m["fqT"] = np.ascontiguousarray(
            full[b, h * SH:(h + 1) * SH, :].T).astype(bf16)
        in_maps.append(m)
    return in_maps


def get_module():
    if "nc" not in _cache:
        _cache["nc"] = _build_module()
    return _cache["nc"]


def run_cores(in_maps):
    from concourse.bass_utils import run_bass_kernel_spmd
    nc = get_module()
    return run_bass_kernel_spmd(nc, in_maps, list(range(N_CORES))).results


def kernel(full, sw1, sb1, sw2, sb2, mw1, mb1, mw2, mb2, mw3, mb3):
    in_maps = _host_inputs(full, sw1, sb1, sw2, sb2, mw1, mb1, mw2, mb2,
                           mw3, mb3)
    results = run_cores(in_maps)
    out = np.empty((B, S, D), dtype=np.float32)
    for c in range(N_CORES):
        b, h = c // 2, c % 2
        out[b, h * SH:(h + 1) * SH, :] = results[c]["outT"].T
    return out


# revision 55
# speedup vs baseline: 1.0035x; 1.0035x over previous
"""Trainium2 Bass kernel for nn_AttentionApproximator (sparse_attention).

Math (per batch b):
  scores = relu(full @ sw1 + sb1) @ sw2 (+ sb2, rank-irrelevant)   [S]
  top_idx = top_k(scores, k=204)                                    (set only)
  sel     = full[top_idx]                                           [k, d]
  q_part  = full @ mw1[:d]                                          [S, 64]
  kvb     = sel @ (mw1[d:2d] + mw1[2d:]) + mb1                      [k, 64]
  h1      = relu(q_part[s] + kvb[j])                                [S, k, 64]
  h2      = relu(h1 @ mw2 + mb2)                                    [S, k, 32]
  out     = mean_j(h2) @ mw3 + mb3                                  [S, d]
        (the mw3 matmul commutes with the j-mean)

Device strategy (8 cores, SPMD): core c handles batch b=c//2, query rows
h=c%2 (1024 of 2048).  Top-k via exact ranks (rank_i = #{j: s_j > s_i});
rank doubles as the compaction slot, gather is a one-hot matmul.

Perf structure vs the reference implementation:
  - scores/bc matmuls run as float32r (1 cycle/row) via bitcast; the
    broadcast bc[p,j]=s_j comes directly out of a tiled-sw2 matmul, so both
    score layouts derive from one tensor (comparison consistency).
  - rank chunks are split across DVE (is_gt+accum) and ACT (Sign+accum);
    one-hots on GPSIMD (trailing ones on DVE); kv copies ride the rank
    window split DVE/ACT (GPSIMD cannot touch PSUM).
  - main loop: h1 = relu(qT2 + kvb) on DVE in 4x perf mode (all-bf16,
    all-SBUF); mw2 in bf16; h2 relu written as fp8e4 into per-supergroup
    [128, 2*1024] tiles (split ACT/DVE; some h1 pairs on GPSIMD); the mw3
    contraction runs as fp8 DoubleRow matmuls covering 8 tokens each
    (0.5 cyc/row), with per-copy dither quantization of mw3.
  - w4 emission is delayed one supergroup so the in-order PE queue never
    stalls waiting for the h2 relu of the newest group.
"""

import os
from contextlib import ExitStack

import numpy as np

B, S, D = 4, 2048, 16
DA = D + 1               # augmented with ones row
K = 204                  # top-k  (int(2048*0.1))
KP = K // 2              # 102 slot-pairs
H1 = 64
H2 = 32
SH = S // 2              # 1024 query rows per core
NCH = S // 128           # 16 token chunks
N_CORES = 8
NG = KP // 2             # 51 groups of 4 tokens
MW3_SCALE = 16.0         # host-side mw3 prescale (exact power of 2)
INV_K = float(np.float32(1.0) / np.float32(K)) / MW3_SCALE

# engine split knobs
RANK_D = int(os.environ.get("KERNEL_RANK_D", "11"))  # DVE rank chunks
RANK_A = int(os.environ.get("KERNEL_RANK_A", "5"))   # ACT rank chunks (rest
# would go to GPSIMD, but walrus rejects accum_out on Pool -- keep D+A=16)
H2_SPLIT = int(os.environ.get("KERNEL_H2_SPLIT", "26"))  # h2 groups split ACT+DVE
H1_POOL = int(os.environ.get("KERNEL_H1_POOL", "16"))  # h1 pairs on GPSIMD
# DoubleRow mw2 is dead on TRN2: the ISA requires DR matmul dst base
# partition 0 (s3d3_mm_valid_dst_partition), so the second token-pair
# (rows 64:128) cannot be produced by a DR matmul. Keep 0.
N_DR = int(os.environ.get("KERNEL_N_DR", "0"))
RANK_D = min(RANK_D, NCH)
RANK_A = NCH - RANK_D

_cache = {}


def _build_module():
    import concourse.mybir as mybir
    import concourse.tile as tile
    from concourse import bacc

    fp32 = mybir.dt.float32
    f32r = mybir.dt.float32r
    bf16 = mybir.dt.bfloat16
    fp8 = mybir.dt.float8e4
    Alu = mybir.AluOpType
    Act = mybir.ActivationFunctionType
    DR = mybir.MatmulPerfMode.DoubleRow

    nc = bacc.Bacc("TRN2", target_bir_lowering=False, debug=False,
                   num_devices=N_CORES)

    # ---- DRAM I/O (order below = DMA issue order; HWDGE is serial) ----
    # pksc: [0:17,0:32]=sw1a | [0:32,32:160]=sw2 tiled 128x | [17,0]=1.0
    d_pksc = nc.dram_tensor("pksc", [32, 160], f32r, kind="ExternalInput").ap()
    d_fbT = nc.dram_tensor("fbT", [DA, S], f32r, kind="ExternalInput").ap()
    d_fqT = nc.dram_tensor("fqT", [D, SH], bf16, kind="ExternalInput").ap()
    d_fbTb = nc.dram_tensor("fbTb", [DA, S], bf16, kind="ExternalInput").ap()
    # pkf: cols 0:204 iota (even slots 0:102, odd 102:204) | 204 mb24 | 205 mb3f
    d_pkf = nc.dram_tensor("pkf", [128, 208], fp32, kind="ExternalInput").ap()
    # pkb: cols 0:64 bdmw2 | [0:16,64:128] wq | [0:17,128:192] wkv
    d_pkb = nc.dram_tensor("pkb", [128, 192], bf16, kind="ExternalInput").ap()
    # pk8: 8 dither-quantized copies of mw3*16 laid out as [128 = 4 rank
    # blocks x 32, 2 DR k-tiles, 16]; per-weight the copies alternate
    # between the two nearest fp8 values so the tiled copies average out
    # the systematic quantization error
    d_pk8 = nc.dram_tensor("pk8", [128, 96], fp8, kind="ExternalInput").ap()

    d_outT = nc.dram_tensor("outT", [D, SH], fp32, kind="ExternalOutput").ap()

    # round-robin the rank chunks through the D/A/P multiset
    rank_asgn = []
    pool_left = {"D": RANK_D, "A": RANK_A, "P": NCH - RANK_D - RANK_A}
    cyc = ["D", "A", "P"]
    ci = 0
    for c in range(NCH):
        for _ in range(3):
            e = cyc[ci % 3]
            ci += 1
            if pool_left[e] > 0:
                pool_left[e] -= 1
                rank_asgn.append(e)
                break
        else:
            rank_asgn.append("D")

    # h2 engine per group: "S" = split halves across ACT+DVE, else ACT
    # (GPSIMD cannot read PSUM so h2 is ACT/DVE only)
    h2_asgn = []
    accS = 0.0
    for g in range(NG):
        accS += H2_SPLIT / NG
        if accS >= 1.0:
            accS -= 1.0
            h2_asgn.append("S")
        else:
            h2_asgn.append("A")
    # DR groups (fp8 DoubleRow mw2): spread evenly, never the tail group
    dr_asgn = [False] * NG
    accR = 0.0
    for g in range(NG - 1):
        accR += N_DR / (NG - 1)
        if accR >= 1.0:
            accR -= 1.0
            dr_asgn[g] = True
    # h1 pair engine: GPSIMD takes fp8 (DR-group) pairs first, then bf16
    # pairs, up to H1_POOL; the rest on DVE
    fp8_pairs = [p for p in range(KP) if dr_asgn[p // 2]]
    bf_pairs = [p for p in range(KP) if not dr_asgn[p // 2]]
    pool_set = set((fp8_pairs + bf_pairs)[:H1_POOL])
    h1_asgn = ["P" if p in pool_set else "D" for p in range(KP)]

    with tile.TileContext(nc) as tc:
        with (
            ExitStack() as es,
            tc.tile_pool(name="const", bufs=1) as cpool,
            tc.tile_pool(name="sel", bufs=1) as spool,
            tc.tile_pool(name="scrD", bufs=2) as scrDp,
            tc.tile_pool(name="scrA", bufs=2) as scrAp,
            tc.tile_pool(name="scrP", bufs=2) as scrPp,
            tc.tile_pool(name="h1p", bufs=4) as h1pool,
            tc.tile_pool(name="h2p", bufs=3) as h2pool,
        ):
            # ---- input DMAs (critical ones first) ----
            sb_pksc = cpool.tile([32, 160], f32r)
            nc.sync.dma_start(sb_pksc[:], d_pksc[:])
            sb_fbT = cpool.tile([DA, S], f32r)
            nc.sync.dma_start(sb_fbT[:], d_fbT[:])
            sb_fqT = cpool.tile([D, SH], bf16)
            nc.sync.dma_start(sb_fqT[:], d_fqT[:])
            sb_fbTb = cpool.tile([DA, S], bf16)
            nc.sync.dma_start(sb_fbTb[:], d_fbTb[:])
            sb_pkf = cpool.tile([128, 208], fp32)
            nc.sync.dma_start(sb_pkf[:], d_pkf[:])
            sb_pkb = cpool.tile([128, 192], bf16)
            nc.sync.dma_start(sb_pkb[:], d_pkb[:])
            sb_pk8 = cpool.tile([128, 96], fp8)
            nc.sync.dma_start(sb_pk8[:], d_pk8[:])

            sb_sw1a = sb_pksc[0:DA, 0:32]
            sb_sw2t = sb_pksc[0:H2, 32:160]
            sb_one = sb_pkf[0:1, 206:207]
            sb_iota = sb_pkf[:, 0:K]
            sb_mb24 = sb_pkf[:, 204:205]
            sb_mb3f = sb_pkf[0:D, 205:206]
            sb_bdmw2 = sb_pkb[:, 0:H1]
            sb_wq = sb_pkb[0:D, 64:128]
            sb_wkv = sb_pkb[0:DA, 128:192]

            # ---- PE warmup while input DMAs stream ----
            with tc.tile_pool(name="pswarm", bufs=1, space="PSUM") as pW:
                ps_w = pW.tile([32, 160], fp32)
                nc.tensor.matmul(ps_w[:], sb_pksc[0:32, 0:32],
                                 sb_pksc[0:32, :], start=True, stop=True)

            # ---- stage A: scores hidden layer + bc broadcast ----
            # bc[p, j] = s_j for every partition p, via sw2-tiled lhsT; the
            # single bc tensor is the source of truth for all score compares.
            sb_bc = spool.tile([128, S], fp32)
            sb_Hs = []
            with tc.tile_pool(name="psA", bufs=4, space="PSUM") as pA, \
                 tc.tile_pool(name="psBC", bufs=4, space="PSUM") as pBC:
                for n in range(4):
                    sl = slice(n * 512, (n + 1) * 512)
                    ps_Hn = pA.tile([H2, 512], fp32, tag="H")
                    nc.tensor.matmul(ps_Hn[:], sb_sw1a, sb_fbT[:, sl],
                                     start=True, stop=True)
                    sb_Hn = spool.tile([H2, 512], f32r, tag=f"sbH{n}")
                    if n % 2 == 0:
                        nc.scalar.activation(sb_Hn[:], ps_Hn[:], Act.Relu)
                    else:
                        nc.vector.tensor_scalar(sb_Hn[:], ps_Hn[:], 0.0, None,
                                                Alu.max)
                    sb_Hs.append(sb_Hn)
                    ps_bcn = pBC.tile([128, 512], fp32, tag="bc")
                    nc.tensor.matmul(ps_bcn[:], sb_sw2t, sb_Hn[:],
                                     start=True, stop=True)
                    if n % 2 == 1:
                        nc.vector.tensor_copy(sb_bc[:, sl], ps_bcn[:])
                    else:
                        nc.scalar.copy(sb_bc[:, sl], ps_bcn[:])

            # ---- stage A2: token-major scores via PE transposes ----
            with tc.tile_pool(name="psA2", bufs=1, space="PSUM") as pA2:
                ps_stok = pA2.tile([128, NCH], fp32)
                for c in range(NCH):
                    nc.tensor.transpose(
                        ps_stok[:, c:c + 1],
                        sb_bc[0:1, c * 128:(c + 1) * 128], sb_one)
                sb_stok = spool.tile([128, NCH], fp32)
                nc.vector.tensor_copy(sb_stok[:], ps_stok[:])


            # ---- stage C1: kv for all tokens (bf16; GPSIMD copies) ----
            es_c = ExitStack()
            pC = es_c.enter_context(tc.tile_pool(name="psC", bufs=2,
                                                 space="PSUM"))
            pCs = es_c.enter_context(tc.tile_pool(name="psCsel", bufs=1,
                                                  space="PSUM"))
            sb_kvtoks = []
            kv_copy_jobs = []
            for c in range(NCH):
                ps_kv = pC.tile([128, H1], fp32, tag="kvtok")
                nc.tensor.matmul(ps_kv[:],
                                 sb_fbTb[:, c * 128:(c + 1) * 128],
                                 sb_wkv, start=True, stop=True)
                sb_kvc = spool.tile([128, H1], bf16, tag=f"kvtok{c}")
                kv_copy_jobs.append((sb_kvc, ps_kv))
                sb_kvtoks.append(sb_kvc)

            # ---- stage D: query part stacked twice [128, SH] bf16 ----
            # (matmuls now; the ACT copy is emitted after ACT's rank chunks)
            es_d = ExitStack()
            pD = es_d.enter_context(tc.tile_pool(name="psD", bufs=1,
                                                 space="PSUM"))
            ps_q = pD.tile([128, SH], fp32)
            for half in range(2):
                rows = slice(half * H1, (half + 1) * H1)
                for n in range(2):
                    sl = slice(n * 512, (n + 1) * 512)
                    nc.tensor.matmul(ps_q[rows, sl], sb_wq,
                                     sb_fqT[:, sl], start=True, stop=True)
            sb_qT2 = cpool.tile([128, SH], bf16)

            # ---- stage B: exact ranks, 3-engine split; kv copies ride in
            # the DVE stream between rank chunks (no rank dependency) ----
            sb_ranks = []
            for c in range(NCH):
                rk = spool.tile([128, 1], fp32, tag=f"rank{c}")
                sb_ranks.append(rk)
            kv_iter = iter(kv_copy_jobs)
            kv_per_d = (len(kv_copy_jobs) + RANK_D - 1) // max(RANK_D, 1)
            kv_ct = [0]
            for c in range(NCH):
                e = rank_asgn[c]
                if e == "A":
                    scr = scrAp.tile([128, S], fp32, tag="scrA")
                    rsgn = spool.tile([128, 1], fp32, tag=f"rsgn{c}")
                    nc.scalar.activation(scr[:], sb_bc[:], Act.Sign,
                                         bias=sb_stok[:, c:c + 1],
                                         scale=-1.0, accum_out=rsgn[:])
                    nc.vector.tensor_scalar(sb_ranks[c][:], rsgn[:], -0.5,
                                            1023.5, Alu.mult, Alu.add)
                elif e == "D":
                    scr = scrDp.tile([128, S], fp32, tag="scrD")
                    nc.vector.tensor_scalar(
                        scr[:], sb_bc[:], sb_stok[:, c:c + 1], 0.0,
                        Alu.is_gt, Alu.add, accum_out=sb_ranks[c][:])
                    for _ in range(kv_per_d):
                        job = next(kv_iter, None)
                        if job is not None:
                            nc.vector.tensor_copy(job[0][:], job[1][:])
                else:
                    scr = scrPp.tile([128, S], fp32, tag="scrP")
                    nc.gpsimd.tensor_scalar(
                        scr[:], sb_bc[:], sb_stok[:, c:c + 1], 0.0,
                        Alu.is_gt, Alu.add, accum_out=sb_ranks[c][:])
            for job in kv_iter:
                nc.vector.tensor_copy(job[0][:], job[1][:])
            # qT2 copy rides after ACT's rank chunks
            nc.scalar.copy(sb_qT2[:], ps_q[:])
            es_d.close()

            # ---- stage C2: one-hot gather of the top-K kv rows (bf16) ----
            ps_kvsel_full = pCs.tile([128, 512], fp32)   # 1 bank
            ps_kvsel = ps_kvsel_full[:, 0:KP]
            for c in range(NCH):
                oh = scrPp.tile([128, K], bf16, tag="oh")
                nc.gpsimd.tensor_scalar(oh[:], sb_iota, sb_ranks[c][:], None,
                                        Alu.is_equal)
                kvch = sb_kvtoks[c][:]
                nc.tensor.matmul(ps_kvsel[0:H1, :], kvch, oh[:, 0:KP],
                                 start=(c == 0), stop=False,
                                 skip_group_check=True)
                nc.tensor.matmul(ps_kvsel[H1:128, :], kvch, oh[:, KP:K],
                                 start=(c == 0), stop=(c == NCH - 1),
                                 skip_group_check=True)
            sb_kvb2a = spool.tile([128, 51], fp32)
            nc.vector.tensor_copy(sb_kvb2a[:], ps_kvsel[:, 0:51])
            sb_kvb2b = spool.tile([128, KP - 51], fp32)
            nc.vector.tensor_copy(sb_kvb2b[:], ps_kvsel[:, 51:KP])
            es_c.close()

            def kvb_col(p):
                return (sb_kvb2a[:, p:p + 1] if p < 51
                        else sb_kvb2b[:, p - 51:p - 50])

            pmain = es.enter_context(
                tc.tile_pool(name="main_psum", bufs=3, space="PSUM"))
            pout = es.enter_context(
                tc.tile_pool(name="out_psum", bufs=1, space="PSUM"))

            # ---- stage E: main pairwise loop ----
            # Group g covers tokens 4g..4g+3 (2 slot-pairs).  h2 of two
            # consecutive groups shares one [128, 2048] fp8 supertile whose
            # halves are the DoubleRow k-tiles of the mw3 contraction.
            ps_out = pout.tile([D, SH], fp32, tag="outacc")
            lhsT_dr = sb_pk8[:, 0:32].rearrange("p (k m) -> p k m", k=2)
            sup = None
            pending_w4 = []
            first_w4 = [True]
            g_order = list(range(NG))

            def emit_w4(sup_t, last=False):
                rhs3 = sup_t[:, 0:2 * SH].rearrange("p (k q) -> p k q", k=2)
                for half in range(2):
                    sl = slice(half * 512, (half + 1) * 512)
                    nc.tensor.matmul(ps_out[:, sl], lhsT_dr, rhs3[:, :, sl],
                                     perf_mode=DR, start=first_w4[0],
                                     stop=last and half == 1,
                                     skip_group_check=True)
                first_w4[0] = False

            nreg = [0]
            for gi, g in enumerate(g_order):
                par = nreg[0] % 2
                if g != NG - 1:
                    nreg[0] += 1
                    if par == 0:
                        sup = h2pool.tile([128, 2 * SH], fp8, tag="sup")
                ps_h = pmain.tile([128, SH], fp32, tag="hps")
                for half in range(2):
                    p = 2 * g + half
                    h1t = h1pool.tile([128, SH], bf16, tag="h1")
                    if h1_asgn[p] == "P":
                        nc.gpsimd.tensor_scalar(h1t[:], sb_qT2[:],
                                                kvb_col(p), 0.0,
                                                Alu.add, Alu.max)
                    else:
                        nc.vector.tensor_scalar(h1t[:], sb_qT2[:],
                                                kvb_col(p), 0.0,
                                                Alu.add, Alu.max)
                    rows = slice(half * H1, (half + 1) * H1)
                    for n in range(2):
                        sl = slice(n * 512, (n + 1) * 512)
                        nc.tensor.matmul(ps_h[rows, sl], sb_bdmw2,
                                         h1t[:, sl], start=True, stop=True)
                if g == NG - 1:
                    # odd 51st group: fp8 tail
                    tail = h2pool.tile([128, SH], fp8, tag="tail")
                    h2dst = tail[:]
                    split = False
                else:
                    h2dst = sup[:, par * SH:(par + 1) * SH]
                    split = h2_asgn[g] != "A"
                if not split:
                    nc.scalar.activation(h2dst, ps_h[:], Act.Relu,
                                         bias=sb_mb24)
                else:
                    nc.scalar.activation(h2dst[:, 0:512], ps_h[:, 0:512],
                                         Act.Relu, bias=sb_mb24)
                    nc.vector.tensor_scalar(h2dst[:, 512:SH],
                                            ps_h[:, 512:SH], sb_mb24, 0.0,
                                            Alu.add, Alu.max)
                if g != NG - 1 and par == 1:
                    pending_w4.append(sup)
                    if len(pending_w4) > 1:
                        emit_w4(pending_w4.pop(0))
            for s_t in pending_w4:
                emit_w4(s_t)
            # tail group: two zero-ktile DR matmuls — the rhs k-tiles are the
            # two query-halves of the tail tile; the unused half multiplies
            # zero weights (pk8 cols 32:96 hold [w3|0] and [0|w3])
            tail3 = tail[:, 0:SH].rearrange("p (k q) -> p k q", k=2)
            lhsT_t0 = sb_pk8[:, 32:64].rearrange("p (k m) -> p k m", k=2)
            lhsT_t1 = sb_pk8[:, 64:96].rearrange("p (k m) -> p k m", k=2)
            nc.tensor.matmul(ps_out[:, 0:512], lhsT_t0, tail3,
                             perf_mode=DR, start=False, stop=False,
                             skip_group_check=True)
            nc.tensor.matmul(ps_out[:, 512:SH], lhsT_t1, tail3,
                             perf_mode=DR, start=False, stop=True,
                             skip_group_check=True)

            # ---- stage F: scale + bias + store ----
            sb_out = spool.tile([D, SH], fp32)
            nc.scalar.activation(sb_out[:], ps_out[:], Act.Identity,
                                 bias=sb_mb3f, scale=INV_K)
            nc.sync.dma_start(d_outT[:], sb_out[:])

    nc.compile()
    return nc


def _host_inputs(full, sw1, sb1, sw2, sb2, mw1, mb1, mw2, mb2, mw3, mb3):
    """Build the 8 per-core input maps (host-side sharding + layout prep)."""
    import ml_dtypes
    f32 = np.float32
    bf16 = ml_dtypes.bfloat16
    fp8 = ml_dtypes.float8_e4m3
    full = np.asarray(full, dtype=f32)
    ones_row = np.ones((1, S), dtype=f32)

    pksc = np.zeros((32, 160), dtype=f32)
    pksc[0:DA, 0:32] = np.concatenate(
        [np.asarray(sw1, dtype=f32), np.asarray(sb1, dtype=f32)[None, :]],
        axis=0)
    pksc[0:H2, 32:160] = np.tile(np.asarray(sw2, dtype=f32).reshape(H2, 1),
                                 (1, 128))

    pkf = np.zeros((128, 208), dtype=f32)
    pkf[0, 206] = 1.0
    pkf[:, 0:KP] = np.arange(0, K, 2, dtype=f32)[None, :]
    pkf[:, KP:K] = np.arange(1, K, 2, dtype=f32)[None, :]
    pkf[:, 204] = np.tile(np.asarray(mb2, dtype=f32), 4)
    pkf[:, 205] = np.tile(np.asarray(mb3, dtype=f32), 8)

    pkb = np.zeros((128, 192), dtype=f32)
    pkb[0:H1, 0:H2] = mw2
    pkb[H1:128, H2:H1] = mw2
    pkb[0:D, 64:128] = mw1[:D]
    pkb[0:DA, 128:192] = np.concatenate(
        [np.asarray(mw1[D:2 * D] + mw1[2 * D:], dtype=f32),
         np.asarray(mb1, dtype=f32)[None, :]], axis=0)
    pkb = pkb.astype(bf16)

    # 8 dither-quantized copies of mw3*16: copy (block b, ktile i) at
    # pk8[32b:32b+32, 16i:16i+16]; per weight, copies alternate between
    # the two fp8 neighbours bracketing the true value so that the
    # average over copies tracks it to ~1/16 ulp
    W = np.asarray(mw3, dtype=f32) * MW3_SCALE            # [32, 16]
    fp8_vals = np.sort(np.unique(
        np.arange(256, dtype=np.uint8).view(fp8).astype(f32)))
    fp8_vals = fp8_vals[np.isfinite(fp8_vals)]
    lo_idx = np.searchsorted(fp8_vals, W, side="right") - 1
    lo = fp8_vals[np.clip(lo_idx, 0, len(fp8_vals) - 1)]
    hi = fp8_vals[np.clip(lo_idx + 1, 0, len(fp8_vals) - 1)]
    denom = np.where(hi > lo, hi - lo, 1.0)
    n_hi = np.round((W - lo) / denom * 8.0)
    pk8 = np.zeros((128, 96), dtype=f32)
    for b in range(4):
        for i in range(2):
            c = 2 * b + i
            cpy = np.where(c < n_hi, hi, lo)
            pk8[32 * b:32 * b + 32, 16 * i:16 * i + 16] = cpy
            # tail lhsTs: [w3|0] at cols 32:64, [0|w3] at 64:96
            pk8[32 * b:32 * b + 32, 32 + 16 * i:48 + 16 * i] = (
                cpy if i == 0 else 0.0)
            pk8[32 * b:32 * b + 32, 64 + 16 * i:80 + 16 * i] = (
                0.0 if i == 0 else cpy)
    pk8 = pk8.astype(fp8)

    shared = dict(pksc=pksc, pkf=pkf, pkb=pkb, pk8=pk8)
    in_maps = []
    for c in range(N_CORES):
        b, h = c // 2, c % 2
        fbT = np.concatenate(
            [np.ascontiguousarray(full[b].T), ones_row], axis=0)
        m = dict(shared)
        m["fbT"] = fbT
        m["fbTb"] = fbT.astype(bf16)
        m["fqT"] = np.ascontiguousarray(
            full[b, h * SH:(h + 1) * SH, :].T).astype(bf16)
        in_maps.append(m)
    return in_maps


def get_module():
    if "nc" not in _cache:
        _cache["nc"] = _build_module()
    return _cache["nc"]


def run_cores(in_maps):
    from concourse.bass_utils import run_bass_kernel_spmd
    nc = get_module()
    return run_bass_kernel_spmd(nc, in_maps, list(range(N_CORES))).results


def kernel(full, sw1, sb1, sw2, sb2, mw1, mb1, mw2, mb2, mw3, mb3):
    in_maps = _host_inputs(full, sw1, sb1, sw2, sb2, mw1, mb1, mw2, mb2,
                           mw3, mb3)
    results = run_cores(in_maps)
    out = np.empty((B, S, D), dtype=np.float32)
    for c in range(N_CORES):
        b, h = c // 2, c % 2
        out[b, h * SH:(h + 1) * SH, :] = results[c]["outT"].T
    return out


# revision 60
# speedup vs baseline: 1.0094x; 1.0059x over previous
"""Trainium2 Bass kernel for nn_AttentionApproximator (sparse_attention).

Math (per batch b):
  scores = relu(full @ sw1 + sb1) @ sw2 (+ sb2, rank-irrelevant)   [S]
  top_idx = top_k(scores, k=204)                                    (set only)
  sel     = full[top_idx]                                           [k, d]
  q_part  = full @ mw1[:d]                                          [S, 64]
  kvb     = sel @ (mw1[d:2d] + mw1[2d:]) + mb1                      [k, 64]
  h1      = relu(q_part[s] + kvb[j])                                [S, k, 64]
  h2      = relu(h1 @ mw2 + mb2)                                    [S, k, 32]
  out     = mean_j(h2) @ mw3 + mb3                                  [S, d]
        (the mw3 matmul commutes with the j-mean)

Device strategy (8 cores, SPMD): core c handles batch b=c//2, query rows
h=c%2 (1024 of 2048).  Top-k via exact ranks (rank_i = #{j: s_j > s_i});
rank doubles as the compaction slot, gather is a one-hot matmul.

Perf structure vs the reference implementation:
  - scores/bc matmuls run as float32r (1 cycle/row) via bitcast; the
    broadcast bc[p,j]=s_j comes directly out of a tiled-sw2 matmul, so both
    score layouts derive from one tensor (comparison consistency).
  - rank chunks are split across DVE (is_gt+accum) and ACT (Sign+accum);
    one-hots on GPSIMD (trailing ones on DVE); kv copies ride the rank
    window split DVE/ACT (GPSIMD cannot touch PSUM).
  - main loop: h1 = relu(qT2 + kvb) on DVE in 4x perf mode (all-bf16,
    all-SBUF); mw2 in bf16; h2 relu written as fp8e4 into per-supergroup
    [128, 2*1024] tiles (split ACT/DVE; some h1 pairs on GPSIMD); the mw3
    contraction runs as fp8 DoubleRow matmuls covering 8 tokens each
    (0.5 cyc/row), with per-copy dither quantization of mw3.
  - w4 emission is delayed one supergroup so the in-order PE queue never
    stalls waiting for the h2 relu of the newest group.
"""

import os
from contextlib import ExitStack

import numpy as np

B, S, D = 4, 2048, 16
DA = D + 1               # augmented with ones row
K = 204                  # top-k  (int(2048*0.1))
KP = K // 2              # 102 slot-pairs
H1 = 64
H2 = 32
SH = S // 2              # 1024 query rows per core
NCH = S // 128           # 16 token chunks
N_CORES = 8
NG = KP // 2             # 51 groups of 4 tokens
MW3_SCALE = 16.0         # host-side mw3 prescale (exact power of 2)
INV_K = float(np.float32(1.0) / np.float32(K)) / MW3_SCALE

# engine split knobs
RANK_D = int(os.environ.get("KERNEL_RANK_D", "11"))  # DVE rank chunks
RANK_A = int(os.environ.get("KERNEL_RANK_A", "5"))   # ACT rank chunks (rest
# would go to GPSIMD, but walrus rejects accum_out on Pool -- keep D+A=16)
H2_SPLIT = int(os.environ.get("KERNEL_H2_SPLIT", "26"))  # h2 groups split ACT+DVE
H1_POOL = int(os.environ.get("KERNEL_H1_POOL", "16"))  # h1 pairs on GPSIMD
# DoubleRow mw2 is dead on TRN2: the ISA requires DR matmul dst base
# partition 0 (s3d3_mm_valid_dst_partition), so the second token-pair
# (rows 64:128) cannot be produced by a DR matmul. Keep 0.
N_DR = int(os.environ.get("KERNEL_N_DR", "0"))
RANK_D = min(RANK_D, NCH)
RANK_A = NCH - RANK_D

_cache = {}


def _build_module():
    import concourse.mybir as mybir
    import concourse.tile as tile
    from concourse import bacc

    fp32 = mybir.dt.float32
    f32r = mybir.dt.float32r
    bf16 = mybir.dt.bfloat16
    fp8 = mybir.dt.float8e4
    Alu = mybir.AluOpType
    Act = mybir.ActivationFunctionType
    DR = mybir.MatmulPerfMode.DoubleRow

    nc = bacc.Bacc("TRN2", target_bir_lowering=False, debug=False,
                   num_devices=N_CORES)

    # ---- DRAM I/O (order below = DMA issue order; HWDGE is serial) ----
    # pksc: [0:17,0:32]=sw1a | [0:32,32:160]=sw2 tiled 128x | [17,0]=1.0
    d_pksc = nc.dram_tensor("pksc", [32, 160], f32r, kind="ExternalInput").ap()
    d_fbT = nc.dram_tensor("fbT", [DA, S], f32r, kind="ExternalInput").ap()
    d_fqT = nc.dram_tensor("fqT", [D, SH], bf16, kind="ExternalInput").ap()
    d_fbTb = nc.dram_tensor("fbTb", [DA, S], bf16, kind="ExternalInput").ap()
    # pkf: cols 0:204 iota (even slots 0:102, odd 102:204) | 204 mb24 | 205 mb3f
    d_pkf = nc.dram_tensor("pkf", [128, 208], fp32, kind="ExternalInput").ap()
    # pkb: cols 0:64 bdmw2 | [0:16,64:128] wq | [0:17,128:192] wkv
    d_pkb = nc.dram_tensor("pkb", [128, 192], bf16, kind="ExternalInput").ap()
    # pk8: 8 dither-quantized copies of mw3*16 laid out as [128 = 4 rank
    # blocks x 32, 2 DR k-tiles, 16]; per-weight the copies alternate
    # between the two nearest fp8 values so the tiled copies average out
    # the systematic quantization error
    d_pk8 = nc.dram_tensor("pk8", [128, 96], fp8, kind="ExternalInput").ap()

    d_outT = nc.dram_tensor("outT", [D, SH], fp32, kind="ExternalOutput").ap()

    # round-robin the rank chunks through the D/A/P multiset
    rank_asgn = []
    pool_left = {"D": RANK_D, "A": RANK_A, "P": NCH - RANK_D - RANK_A}
    cyc = ["D", "A", "P"]
    ci = 0
    for c in range(NCH):
        for _ in range(3):
            e = cyc[ci % 3]
            ci += 1
            if pool_left[e] > 0:
                pool_left[e] -= 1
                rank_asgn.append(e)
                break
        else:
            rank_asgn.append("D")

    # h2 engine per group: "S" = split halves across ACT+DVE, else ACT
    # (GPSIMD cannot read PSUM so h2 is ACT/DVE only)
    h2_asgn = []
    accS = 0.0
    for g in range(NG):
        accS += H2_SPLIT / NG
        if accS >= 1.0:
            accS -= 1.0
            h2_asgn.append("S")
        else:
            h2_asgn.append("A")
    # DR groups (fp8 DoubleRow mw2): spread evenly, never the tail group
    dr_asgn = [False] * NG
    accR = 0.0
    for g in range(NG - 1):
        accR += N_DR / (NG - 1)
        if accR >= 1.0:
            accR -= 1.0
            dr_asgn[g] = True
    # h1 pair engine: GPSIMD takes fp8 (DR-group) pairs first, then bf16
    # pairs, up to H1_POOL; the rest on DVE
    fp8_pairs = [p for p in range(KP) if dr_asgn[p // 2]]
    bf_pairs = [p for p in range(KP) if not dr_asgn[p // 2]]
    pool_set = set((fp8_pairs + bf_pairs)[:H1_POOL])
    h1_asgn = ["P" if p in pool_set else "D" for p in range(KP)]

    with tile.TileContext(nc) as tc:
        with (
            ExitStack() as es,
            tc.tile_pool(name="const", bufs=1) as cpool,
            tc.tile_pool(name="sel", bufs=1) as spool,
            tc.tile_pool(name="scrD", bufs=2) as scrDp,
            tc.tile_pool(name="scrA", bufs=2) as scrAp,
            tc.tile_pool(name="scrP", bufs=2) as scrPp,
            tc.tile_pool(name="h1p", bufs=4) as h1pool,
            tc.tile_pool(name="h2p", bufs=3) as h2pool,
        ):
            # ---- input DMAs (critical ones first) ----
            sb_pksc = cpool.tile([32, 160], f32r)
            nc.sync.dma_start(sb_pksc[:], d_pksc[:])
            sb_fbT = cpool.tile([DA, S], f32r)
            nc.sync.dma_start(sb_fbT[:], d_fbT[:])
            sb_fqT = cpool.tile([D, SH], bf16)
            nc.sync.dma_start(sb_fqT[:], d_fqT[:])
            sb_fbTb = cpool.tile([DA, S], bf16)
            nc.sync.dma_start(sb_fbTb[:], d_fbTb[:])
            sb_pkf = cpool.tile([128, 208], fp32)
            nc.sync.dma_start(sb_pkf[:], d_pkf[:])
            sb_pkb = cpool.tile([128, 192], bf16)
            nc.sync.dma_start(sb_pkb[:], d_pkb[:])
            sb_pk8 = cpool.tile([128, 96], fp8)
            nc.sync.dma_start(sb_pk8[:], d_pk8[:])

            sb_sw1a = sb_pksc[0:DA, 0:32]
            sb_sw2t = sb_pksc[0:H2, 32:160]
            sb_one = sb_pkf[0:1, 206:207]
            sb_iota = sb_pkf[:, 0:K]
            sb_mb24 = sb_pkf[:, 204:205]
            sb_mb3f = sb_pkf[0:D, 205:206]
            sb_bdmw2 = sb_pkb[:, 0:H1]
            sb_wq = sb_pkb[0:D, 64:128]
            sb_wkv = sb_pkb[0:DA, 128:192]

            # ---- PE warmup while input DMAs stream ----
            with tc.tile_pool(name="pswarm", bufs=1, space="PSUM") as pW:
                ps_w = pW.tile([32, 160], fp32)
                nc.tensor.matmul(ps_w[:], sb_pksc[0:32, 0:32],
                                 sb_pksc[0:32, :], start=True, stop=True)

            # ---- stage A: scores hidden layer + bc broadcast ----
            # bc[p, j] = s_j for every partition p, via sw2-tiled lhsT; the
            # single bc tensor is the source of truth for all score compares.
            sb_bc = spool.tile([128, S], fp32)
            sb_Hs = []
            with tc.tile_pool(name="psA", bufs=4, space="PSUM") as pA, \
                 tc.tile_pool(name="psBC", bufs=4, space="PSUM") as pBC:
                for n in range(4):
                    sl = slice(n * 512, (n + 1) * 512)
                    ps_Hn = pA.tile([H2, 512], fp32, tag="H")
                    nc.tensor.matmul(ps_Hn[:], sb_sw1a, sb_fbT[:, sl],
                                     start=True, stop=True)
                    sb_Hn = spool.tile([H2, 512], f32r, tag=f"sbH{n}")
                    if n % 2 == 0:
                        nc.scalar.activation(sb_Hn[:], ps_Hn[:], Act.Relu)
                    else:
                        nc.vector.tensor_scalar(sb_Hn[:], ps_Hn[:], 0.0, None,
                                                Alu.max)
                    sb_Hs.append(sb_Hn)
                    ps_bcn = pBC.tile([128, 512], fp32, tag="bc")
                    nc.tensor.matmul(ps_bcn[:], sb_sw2t, sb_Hn[:],
                                     start=True, stop=True)
                    if n % 2 == 1:
                        nc.vector.tensor_copy(sb_bc[:, sl], ps_bcn[:])
                    else:
                        nc.scalar.copy(sb_bc[:, sl], ps_bcn[:])

            # ---- stage A2: token-major scores via PE transposes ----
            with tc.tile_pool(name="psA2", bufs=1, space="PSUM") as pA2:
                ps_stok = pA2.tile([128, NCH], fp32)
                for c in range(NCH):
                    nc.tensor.transpose(
                        ps_stok[:, c:c + 1],
                        sb_bc[0:1, c * 128:(c + 1) * 128], sb_one)
                sb_stok = spool.tile([128, NCH], fp32)
                nc.vector.tensor_copy(sb_stok[:], ps_stok[:])


            # ---- stage C1: kv for all tokens (bf16; GPSIMD copies) ----
            es_c = ExitStack()
            pC = es_c.enter_context(tc.tile_pool(name="psC", bufs=2,
                                                 space="PSUM"))
            pCs = es_c.enter_context(tc.tile_pool(name="psCsel", bufs=1,
                                                  space="PSUM"))
            sb_kvtoks = []
            kv_copy_jobs = []
            for c in range(NCH):
                ps_kv = pC.tile([128, H1], fp32, tag="kvtok")
                nc.tensor.matmul(ps_kv[:],
                                 sb_fbTb[:, c * 128:(c + 1) * 128],
                                 sb_wkv, start=True, stop=True)
                sb_kvc = spool.tile([128, H1], bf16, tag=f"kvtok{c}")
                kv_copy_jobs.append((sb_kvc, ps_kv))
                sb_kvtoks.append(sb_kvc)

            # ---- stage D: query part stacked twice [128, SH] bf16 ----
            # (matmuls now; the ACT copy is emitted after ACT's rank chunks)
            es_d = ExitStack()
            pD = es_d.enter_context(tc.tile_pool(name="psD", bufs=1,
                                                 space="PSUM"))
            ps_q = pD.tile([128, SH], fp32)
            for half in range(2):
                rows = slice(half * H1, (half + 1) * H1)
                for n in range(2):
                    sl = slice(n * 512, (n + 1) * 512)
                    nc.tensor.matmul(ps_q[rows, sl], sb_wq,
                                     sb_fqT[:, sl], start=True, stop=True)
            sb_qT2 = cpool.tile([128, SH], bf16)

            # ---- stage B: exact ranks, 3-engine split; kv copies ride in
            # the DVE stream between rank chunks (no rank dependency) ----
            sb_ranks = []
            for c in range(NCH):
                rk = spool.tile([128, 1], fp32, tag=f"rank{c}")
                sb_ranks.append(rk)
            kv_iter = iter(kv_copy_jobs)
            kv_per_d = (len(kv_copy_jobs) + RANK_D - 1) // max(RANK_D, 1)
            kv_ct = [0]
            for c in range(NCH):
                e = rank_asgn[c]
                if e == "A":
                    scr = scrAp.tile([128, S], fp32, tag="scrA")
                    rsgn = spool.tile([128, 1], fp32, tag=f"rsgn{c}")
                    nc.scalar.activation(scr[:], sb_bc[:], Act.Sign,
                                         bias=sb_stok[:, c:c + 1],
                                         scale=-1.0, accum_out=rsgn[:])
                    nc.vector.tensor_scalar(sb_ranks[c][:], rsgn[:], -0.5,
                                            1023.5, Alu.mult, Alu.add)
                elif e == "D":
                    scr = scrDp.tile([128, S], fp32, tag="scrD")
                    nc.vector.tensor_scalar(
                        scr[:], sb_bc[:], sb_stok[:, c:c + 1], 0.0,
                        Alu.is_gt, Alu.add, accum_out=sb_ranks[c][:])
                    for _ in range(kv_per_d):
                        job = next(kv_iter, None)
                        if job is not None:
                            nc.vector.tensor_copy(job[0][:], job[1][:])
                else:
                    scr = scrPp.tile([128, S], fp32, tag="scrP")
                    nc.gpsimd.tensor_scalar(
                        scr[:], sb_bc[:], sb_stok[:, c:c + 1], 0.0,
                        Alu.is_gt, Alu.add, accum_out=sb_ranks[c][:])
            for job in kv_iter:
                nc.vector.tensor_copy(job[0][:], job[1][:])
            # qT2 copy rides after ACT's rank chunks
            nc.scalar.copy(sb_qT2[:], ps_q[:])
            es_d.close()

            # ---- stage C2: one-hot gather of the top-K kv rows (bf16) ----
            ps_kvsel_full = pCs.tile([128, 512], fp32)   # 1 bank
            ps_kvsel = ps_kvsel_full[:, 0:KP]
            for c in range(NCH):
                oh = scrPp.tile([128, K], bf16, tag="oh")
                nc.gpsimd.tensor_scalar(oh[:], sb_iota, sb_ranks[c][:], None,
                                        Alu.is_equal)
                kvch = sb_kvtoks[c][:]
                nc.tensor.matmul(ps_kvsel[0:H1, :], kvch, oh[:, 0:KP],
                                 start=(c == 0), stop=False,
                                 skip_group_check=True)
                nc.tensor.matmul(ps_kvsel[H1:128, :], kvch, oh[:, KP:K],
                                 start=(c == 0), stop=(c == NCH - 1),
                                 skip_group_check=True)
            sb_kvb2a = spool.tile([128, 51], fp32)
            nc.vector.tensor_copy(sb_kvb2a[:], ps_kvsel[:, 0:51])
            sb_kvb2b = spool.tile([128, KP - 51], fp32)
            nc.vector.tensor_copy(sb_kvb2b[:], ps_kvsel[:, 51:KP])
            es_c.close()

            def kvb_col(p):
                return (sb_kvb2a[:, p:p + 1] if p < 51
                        else sb_kvb2b[:, p - 51:p - 50])

            pmain = es.enter_context(
                tc.tile_pool(name="main_psum", bufs=3, space="PSUM"))
            pout = es.enter_context(
                tc.tile_pool(name="out_psum", bufs=1, space="PSUM"))

            # ---- stage E: main pairwise loop ----
            # Group g covers tokens 4g..4g+3 (2 slot-pairs).  h2 of two
            # consecutive groups shares one [128, 2048] fp8 supertile whose
            # halves are the DoubleRow k-tiles of the mw3 contraction.
            ps_out = pout.tile([D, SH], fp32, tag="outacc")
            lhsT_dr = sb_pk8[:, 0:32].rearrange("p (k m) -> p k m", k=2)
            sup = None
            pending_w4 = []
            first_w4 = [True]
            g_order = list(range(NG))

            def emit_w4(sup_t, last=False):
                rhs3 = sup_t[:, 0:2 * SH].rearrange("p (k q) -> p k q", k=2)
                for half in range(2):
                    sl = slice(half * 512, (half + 1) * 512)
                    nc.tensor.matmul(ps_out[:, sl], lhsT_dr, rhs3[:, :, sl],
                                     perf_mode=DR, start=first_w4[0],
                                     stop=last and half == 1,
                                     skip_group_check=True)
                first_w4[0] = False

            nreg = [0]
            for gi, g in enumerate(g_order):
                par = nreg[0] % 2
                if g != NG - 1:
                    nreg[0] += 1
                    if par == 0:
                        sup = h2pool.tile([128, 2 * SH], fp8, tag="sup")
                ps_h = pmain.tile([128, SH], fp32, tag="hps")
                for half in range(2):
                    p = 2 * g + half
                    h1t = h1pool.tile([128, SH], bf16, tag="h1")
                    if h1_asgn[p] == "P":
                        nc.gpsimd.tensor_scalar(h1t[:], sb_qT2[:],
                                                kvb_col(p), 0.0,
                                                Alu.add, Alu.max)
                    else:
                        nc.vector.tensor_scalar(h1t[:], sb_qT2[:],
                                                kvb_col(p), 0.0,
                                                Alu.add, Alu.max)
                    rows = slice(half * H1, (half + 1) * H1)
                    for n in range(2):
                        sl = slice(n * 512, (n + 1) * 512)
                        nc.tensor.matmul(ps_h[rows, sl], sb_bdmw2,
                                         h1t[:, sl], start=True, stop=True)
                if g == NG - 1:
                    # odd 51st group: fp8 tail
                    tail = h2pool.tile([128, SH], fp8, tag="tail")
                    h2dst = tail[:]
                    split = False
                else:
                    h2dst = sup[:, par * SH:(par + 1) * SH]
                    split = h2_asgn[g] != "A"
                if not split:
                    nc.scalar.activation(h2dst, ps_h[:], Act.Relu,
                                         bias=sb_mb24)
                else:
                    nc.scalar.activation(h2dst[:, 0:512], ps_h[:, 0:512],
                                         Act.Relu, bias=sb_mb24)
                    nc.vector.tensor_scalar(h2dst[:, 512:SH],
                                            ps_h[:, 512:SH], sb_mb24, 0.0,
                                            Alu.add, Alu.max)
                if g != NG - 1 and par == 1:
                    pending_w4.append(sup)
                    if len(pending_w4) > 1:
                        emit_w4(pending_w4.pop(0))
            for s_t in pending_w4:
                emit_w4(s_t)
            # tail group: two zero-ktile DR matmuls — the rhs k-tiles are the
            # two query-halves of the tail tile; the unused half multiplies
            # zero weights (pk8 cols 32:96 hold [w3|0] and [0|w3])
            tail3 = tail[:, 0:SH].rearrange("p (k q) -> p k q", k=2)
            lhsT_t0 = sb_pk8[:, 32:64].rearrange("p (k m) -> p k m", k=2)
            lhsT_t1 = sb_pk8[:, 64:96].rearrange("p (k m) -> p k m", k=2)
            nc.tensor.matmul(ps_out[:, 0:512], lhsT_t0, tail3,
                             perf_mode=DR, start=False, stop=False,
                             skip_group_check=True)
            nc.tensor.matmul(ps_out[:, 512:SH], lhsT_t1, tail3,
                             perf_mode=DR, start=False, stop=True,
                             skip_group_check=True)

            # ---- stage F: scale + bias + store ----
            sb_out = spool.tile([D, SH], fp32)
            nc.scalar.activation(sb_out[:], ps_out[:], Act.Identity,
                                 bias=sb_mb3f, scale=INV_K)
            nc.sync.dma_start(d_outT[:], sb_out[:])

    nc.compile()
    return nc


def _host_inputs(full, sw1, sb1, sw2, sb2, mw1, mb1, mw2, mb2, mw3, mb3):
    """Build the 8 per-core input maps (host-side sharding + layout prep)."""
    import ml_dtypes
    f32 = np.float32
    bf16 = ml_dtypes.bfloat16
    fp8 = ml_dtypes.float8_e4m3
    full = np.asarray(full, dtype=f32)
    ones_row = np.ones((1, S), dtype=f32)

    pksc = np.zeros((32, 160), dtype=f32)
    pksc[0:DA, 0:32] = np.concatenate(
        [np.asarray(sw1, dtype=f32), np.asarray(sb1, dtype=f32)[None, :]],
        axis=0)
    pksc[0:H2, 32:160] = np.tile(np.asarray(sw2, dtype=f32).reshape(H2, 1),
                                 (1, 128))

    pkf = np.zeros((128, 208), dtype=f32)
    pkf[0, 206] = 1.0
    pkf[:, 0:KP] = np.arange(0, K, 2, dtype=f32)[None, :]
    pkf[:, KP:K] = np.arange(1, K, 2, dtype=f32)[None, :]
    pkf[:, 204] = np.tile(np.asarray(mb2, dtype=f32), 4)
    pkf[:, 205] = np.tile(np.asarray(mb3, dtype=f32), 8)

    pkb = np.zeros((128, 192), dtype=f32)
    pkb[0:H1, 0:H2] = mw2
    pkb[H1:128, H2:H1] = mw2
    pkb[0:D, 64:128] = mw1[:D]
    pkb[0:DA, 128:192] = np.concatenate(
        [np.asarray(mw1[D:2 * D] + mw1[2 * D:], dtype=f32),
         np.asarray(mb1, dtype=f32)[None, :]], axis=0)
    pkb = pkb.astype(bf16)

    # 8 dither-quantized copies of mw3*16: copy (block b, ktile i) at
    # pk8[32b:32b+32, 16i:16i+16]; per weight, copies alternate between
    # the two fp8 neighbours bracketing the true value so that the
    # average over copies tracks it to ~1/16 ulp
    W = np.asarray(mw3, dtype=f32) * MW3_SCALE            # [32, 16]
    fp8_vals = np.sort(np.unique(
        np.arange(256, dtype=np.uint8).view(fp8).astype(f32)))
    fp8_vals = fp8_vals[np.isfinite(fp8_vals)]
    lo_idx = np.searchsorted(fp8_vals, W, side="right") - 1
    lo = fp8_vals[np.clip(lo_idx, 0, len(fp8_vals) - 1)]
    hi = fp8_vals[np.clip(lo_idx + 1, 0, len(fp8_vals) - 1)]
    denom = np.where(hi > lo, hi - lo, 1.0)
    n_hi = np.round((W - lo) / denom * 8.0)
    pk8 = np.zeros((128, 96), dtype=f32)
    for b in range(4):
        for i in range(2):
            c = 2 * b + i
            cpy = np.where(c < n_hi, hi, lo)
            pk8[32 * b:32 * b + 32, 16 * i:16 * i + 16] = cpy
            # tail lhsTs: [w3|0] at cols 32:64, [0|w3] at 64:96
            pk8[32 * b:32 * b + 32, 32 + 16 * i:48 + 16 * i] = (
                cpy if i == 0 else 0.0)
            pk8[32 * b:32 * b + 32, 64 + 16 * i:80 + 16 * i] = (
                0.0 if i == 0 else cpy)
    pk8 = pk8.astype(fp8)

    shared = dict(pksc=pksc, pkf=pkf, pkb=pkb, pk8=pk8)
    in_maps = []
    for c in range(N_CORES):
        b, h = c // 2, c % 2
        fbT = np.concatenate(
            [np.ascontiguousarray(full[b].T), ones_row], axis=0)
        m = dict(shared)
        m["fbT"] = fbT
        m["fbTb"] = fbT.astype(bf16)
        m["fqT"] = np.ascontiguousarray(
            full[b, h * SH:(h + 1) * SH, :].T).astype(bf16)
        in_maps.append(m)
    return in_maps


def get_module():
    if "nc" not in _cache:
        _cache["nc"] = _build_module()
    return _cache["nc"]


def run_cores(in_maps):
    from concourse.bass_utils import run_bass_kernel_spmd
    nc = get_module()
    return run_bass_kernel_spmd(nc, in_maps, list(range(N_CORES))).results


def kernel(full, sw1, sb1, sw2, sb2, mw1, mb1, mw2, mb2, mw3, mb3):
    in_maps = _host_inputs(full, sw1, sb1, sw2, sb2, mw1, mb1, mw2, mb2,
                           mw3, mb3)
    results = run_cores(in_maps)
    out = np.empty((B, S, D), dtype=np.float32)
    for c in range(N_CORES):
        b, h = c // 2, c % 2
        out[b, h * SH:(h + 1) * SH, :] = results[c]["outT"].T
    return out


# revision 66
# speedup vs baseline: 1.0095x; 1.0001x over previous
"""Trainium2 Bass kernel for nn_AttentionApproximator (sparse_attention).

Math (per batch b):
  scores = relu(full @ sw1 + sb1) @ sw2 (+ sb2, rank-irrelevant)   [S]
  top_idx = top_k(scores, k=204)                                    (set only)
  sel     = full[top_idx]                                           [k, d]
  q_part  = full @ mw1[:d]                                          [S, 64]
  kvb     = sel @ (mw1[d:2d] + mw1[2d:]) + mb1                      [k, 64]
  h1      = relu(q_part[s] + kvb[j])                                [S, k, 64]
  h2      = relu(h1 @ mw2 + mb2)                                    [S, k, 32]
  out     = mean_j(h2) @ mw3 + mb3                                  [S, d]
        (the mw3 matmul commutes with the j-mean)

Device strategy (8 cores, SPMD): core c handles batch b=c//2, query rows
h=c%2 (1024 of 2048).  Top-k via exact ranks (rank_i = #{j: s_j > s_i});
rank doubles as the compaction slot, gather is a one-hot matmul.

Perf structure vs the reference implementation:
  - scores/bc matmuls run as float32r (1 cycle/row) via bitcast; the
    broadcast bc[p,j]=s_j comes directly out of a tiled-sw2 matmul, so both
    score layouts derive from one tensor (comparison consistency).
  - rank chunks are split across DVE (is_gt+accum) and ACT (Sign+accum);
    one-hots on GPSIMD (trailing ones on DVE); kv copies ride the rank
    window split DVE/ACT (GPSIMD cannot touch PSUM).
  - main loop: h1 = relu(qT2 + kvb) on DVE in 4x perf mode (all-bf16,
    all-SBUF); mw2 in bf16; h2 relu written as fp8e4 into per-supergroup
    [128, 2*1024] tiles (split ACT/DVE; some h1 pairs on GPSIMD); the mw3
    contraction runs as fp8 DoubleRow matmuls covering 8 tokens each
    (0.5 cyc/row), with per-copy dither quantization of mw3.
  - w4 emission is delayed one supergroup so the in-order PE queue never
    stalls waiting for the h2 relu of the newest group.
"""

import os
from contextlib import ExitStack

import numpy as np

B, S, D = 4, 2048, 16
DA = D + 1               # augmented with ones row
K = 204                  # top-k  (int(2048*0.1))
KP = K // 2              # 102 slot-pairs
H1 = 64
H2 = 32
SH = S // 2              # 1024 query rows per core
NCH = S // 128           # 16 token chunks
N_CORES = 8
NG = KP // 2             # 51 groups of 4 tokens
MW3_SCALE = 16.0         # host-side mw3 prescale (exact power of 2)
INV_K = float(np.float32(1.0) / np.float32(K)) / MW3_SCALE

# engine split knobs
RANK_D = int(os.environ.get("KERNEL_RANK_D", "11"))  # DVE rank chunks
RANK_A = int(os.environ.get("KERNEL_RANK_A", "5"))   # ACT rank chunks (rest
# would go to GPSIMD, but walrus rejects accum_out on Pool -- keep D+A=16)
H2_SPLIT = int(os.environ.get("KERNEL_H2_SPLIT", "26"))  # h2 groups split ACT+DVE
H1_POOL = int(os.environ.get("KERNEL_H1_POOL", "16"))  # h1 pairs on GPSIMD
# DoubleRow mw2 is dead on TRN2: the ISA requires DR matmul dst base
# partition 0 (s3d3_mm_valid_dst_partition), so the second token-pair
# (rows 64:128) cannot be produced by a DR matmul. Keep 0.
N_DR = int(os.environ.get("KERNEL_N_DR", "0"))
RANK_D = min(RANK_D, NCH)
RANK_A = NCH - RANK_D

_cache = {}


def _build_module():
    import concourse.mybir as mybir
    import concourse.tile as tile
    from concourse import bacc

    fp32 = mybir.dt.float32
    f32r = mybir.dt.float32r
    bf16 = mybir.dt.bfloat16
    fp8 = mybir.dt.float8e4
    Alu = mybir.AluOpType
    Act = mybir.ActivationFunctionType
    DR = mybir.MatmulPerfMode.DoubleRow

    nc = bacc.Bacc("TRN2", target_bir_lowering=False, debug=False,
                   num_devices=N_CORES)

    # ---- DRAM I/O (order below = DMA issue order; HWDGE is serial) ----
    # pksc: [0:17,0:32]=sw1a | [0:32,32:160]=sw2 tiled 128x | [17,0]=1.0
    d_pksc = nc.dram_tensor("pksc", [32, 160], f32r, kind="ExternalInput").ap()
    d_fbT = nc.dram_tensor("fbT", [DA, S], f32r, kind="ExternalInput").ap()
    d_fqT = nc.dram_tensor("fqT", [D, SH], bf16, kind="ExternalInput").ap()
    d_fbTb = nc.dram_tensor("fbTb", [DA, S], bf16, kind="ExternalInput").ap()
    # pkf: cols 0:204 iota (even slots 0:102, odd 102:204) | 204 mb24 | 205 mb3f
    d_pkf = nc.dram_tensor("pkf", [128, 208], fp32, kind="ExternalInput").ap()
    # pkb: cols 0:64 bdmw2 | [0:16,64:128] wq | [0:17,128:192] wkv
    d_pkb = nc.dram_tensor("pkb", [128, 192], bf16, kind="ExternalInput").ap()
    # pk8: 8 dither-quantized copies of mw3*16 laid out as [128 = 4 rank
    # blocks x 32, 2 DR k-tiles, 16]; per-weight the copies alternate
    # between the two nearest fp8 values so the tiled copies average out
    # the systematic quantization error
    d_pk8 = nc.dram_tensor("pk8", [128, 96], fp8, kind="ExternalInput").ap()

    d_outT = nc.dram_tensor("outT", [D, SH], fp32, kind="ExternalOutput").ap()

    # round-robin the rank chunks through the D/A/P multiset
    rank_asgn = []
    pool_left = {"D": RANK_D, "A": RANK_A, "P": NCH - RANK_D - RANK_A}
    cyc = ["D", "A", "P"]
    ci = 0
    for c in range(NCH):
        for _ in range(3):
            e = cyc[ci % 3]
            ci += 1
            if pool_left[e] > 0:
                pool_left[e] -= 1
                rank_asgn.append(e)
                break
        else:
            rank_asgn.append("D")

    # h2 engine per group: "S" = split halves across ACT+DVE, else ACT
    # (GPSIMD cannot read PSUM so h2 is ACT/DVE only)
    h2_asgn = []
    accS = 0.0
    for g in range(NG):
        accS += H2_SPLIT / NG
        if accS >= 1.0:
            accS -= 1.0
            h2_asgn.append("S")
        else:
            h2_asgn.append("A")
    # DR groups (fp8 DoubleRow mw2): spread evenly, never the tail group
    dr_asgn = [False] * NG
    accR = 0.0
    for g in range(NG - 1):
        accR += N_DR / (NG - 1)
        if accR >= 1.0:
            accR -= 1.0
            dr_asgn[g] = True
    # h1 pair engine: GPSIMD takes fp8 (DR-group) pairs first, then bf16
    # pairs, up to H1_POOL; the rest on DVE
    fp8_pairs = [p for p in range(KP) if dr_asgn[p // 2]]
    bf_pairs = [p for p in range(KP) if not dr_asgn[p // 2]]
    pool_set = set((fp8_pairs + bf_pairs)[:H1_POOL])
    h1_asgn = ["P" if p in pool_set else "D" for p in range(KP)]

    with tile.TileContext(nc) as tc:
        with (
            ExitStack() as es,
            tc.tile_pool(name="const", bufs=1) as cpool,
            tc.tile_pool(name="sel", bufs=1) as spool,
            tc.tile_pool(name="scrD", bufs=2) as scrDp,
            tc.tile_pool(name="scrA", bufs=2) as scrAp,
            tc.tile_pool(name="scrP", bufs=2) as scrPp,
            tc.tile_pool(name="h1p", bufs=4) as h1pool,
            tc.tile_pool(name="h2p", bufs=3) as h2pool,
        ):
            # ---- input DMAs (critical ones first) ----
            sb_pksc = cpool.tile([32, 160], f32r)
            nc.sync.dma_start(sb_pksc[:], d_pksc[:])
            sb_fbT = cpool.tile([DA, S], f32r)
            nc.sync.dma_start(sb_fbT[:], d_fbT[:])
            sb_fqT = cpool.tile([D, SH], bf16)
            nc.sync.dma_start(sb_fqT[:], d_fqT[:])
            sb_fbTb = cpool.tile([DA, S], bf16)
            nc.sync.dma_start(sb_fbTb[:], d_fbTb[:])
            sb_pkf = cpool.tile([128, 208], fp32)
            nc.sync.dma_start(sb_pkf[:], d_pkf[:])
            sb_pkb = cpool.tile([128, 192], bf16)
            nc.sync.dma_start(sb_pkb[:], d_pkb[:])
            sb_pk8 = cpool.tile([128, 96], fp8)
            nc.sync.dma_start(sb_pk8[:], d_pk8[:])

            sb_sw1a = sb_pksc[0:DA, 0:32]
            sb_sw2t = sb_pksc[0:H2, 32:160]
            sb_one = sb_pkf[0:1, 206:207]
            sb_iota = sb_pkf[:, 0:K]
            sb_mb24 = sb_pkf[:, 204:205]
            sb_mb3f = sb_pkf[0:D, 205:206]
            sb_bdmw2 = sb_pkb[:, 0:H1]
            sb_wq = sb_pkb[0:D, 64:128]
            sb_wkv = sb_pkb[0:DA, 128:192]

            # ---- PE warmup while input DMAs stream ----
            with tc.tile_pool(name="pswarm", bufs=1, space="PSUM") as pW:
                ps_w = pW.tile([32, 160], fp32)
                nc.tensor.matmul(ps_w[:], sb_pksc[0:32, 0:32],
                                 sb_pksc[0:32, :], start=True, stop=True)

            # ---- stage A: scores hidden layer + bc broadcast ----
            # bc[p, j] = s_j for every partition p, via sw2-tiled lhsT; the
            # single bc tensor is the source of truth for all score compares.
            sb_bc = spool.tile([128, S], fp32)
            sb_Hs = []
            with tc.tile_pool(name="psA", bufs=4, space="PSUM") as pA, \
                 tc.tile_pool(name="psBC", bufs=4, space="PSUM") as pBC:
                for n in range(4):
                    sl = slice(n * 512, (n + 1) * 512)
                    ps_Hn = pA.tile([H2, 512], fp32, tag="H")
                    nc.tensor.matmul(ps_Hn[:], sb_sw1a, sb_fbT[:, sl],
                                     start=True, stop=True)
                    sb_Hn = spool.tile([H2, 512], f32r, tag=f"sbH{n}")
                    if n % 2 == 0:
                        nc.scalar.activation(sb_Hn[:], ps_Hn[:], Act.Relu)
                    else:
                        nc.vector.tensor_scalar(sb_Hn[:], ps_Hn[:], 0.0, None,
                                                Alu.max)
                    sb_Hs.append(sb_Hn)
                    ps_bcn = pBC.tile([128, 512], fp32, tag="bc")
                    nc.tensor.matmul(ps_bcn[:], sb_sw2t, sb_Hn[:],
                                     start=True, stop=True)
                    if n % 2 == 1:
                        nc.vector.tensor_copy(sb_bc[:, sl], ps_bcn[:])
                    else:
                        nc.scalar.copy(sb_bc[:, sl], ps_bcn[:])

            # ---- stage A2: token-major scores via PE transposes ----
            with tc.tile_pool(name="psA2", bufs=1, space="PSUM") as pA2:
                ps_stok = pA2.tile([128, NCH], fp32)
                for c in range(NCH):
                    nc.tensor.transpose(
                        ps_stok[:, c:c + 1],
                        sb_bc[0:1, c * 128:(c + 1) * 128], sb_one)
                sb_stok = spool.tile([128, NCH], fp32)
                nc.scalar.copy(sb_stok[:], ps_stok[:])


            # ---- stage C1: kv for all tokens (bf16; GPSIMD copies) ----
            es_c = ExitStack()
            pC = es_c.enter_context(tc.tile_pool(name="psC", bufs=2,
                                                 space="PSUM"))
            pCs = es_c.enter_context(tc.tile_pool(name="psCsel", bufs=1,
                                                  space="PSUM"))
            sb_kvtoks = []
            kv_copy_jobs = []
            for c in range(NCH):
                ps_kv = pC.tile([128, H1], fp32, tag="kvtok")
                nc.tensor.matmul(ps_kv[:],
                                 sb_fbTb[:, c * 128:(c + 1) * 128],
                                 sb_wkv, start=True, stop=True)
                sb_kvc = spool.tile([128, H1], bf16, tag=f"kvtok{c}")
                kv_copy_jobs.append((sb_kvc, ps_kv))
                sb_kvtoks.append(sb_kvc)

            # ---- stage D: query part stacked twice [128, SH] bf16 ----
            # (matmuls now; the ACT copy is emitted after ACT's rank chunks)
            es_d = ExitStack()
            pD = es_d.enter_context(tc.tile_pool(name="psD", bufs=1,
                                                 space="PSUM"))
            ps_q = pD.tile([128, SH], fp32)
            for half in range(2):
                rows = slice(half * H1, (half + 1) * H1)
                for n in range(2):
                    sl = slice(n * 512, (n + 1) * 512)
                    nc.tensor.matmul(ps_q[rows, sl], sb_wq,
                                     sb_fqT[:, sl], start=True, stop=True)
            sb_qT2 = cpool.tile([128, SH], bf16)

            # ---- stage B: exact ranks, 3-engine split; kv copies ride in
            # the DVE stream between rank chunks (no rank dependency) ----
            sb_ranks = []
            for c in range(NCH):
                rk = spool.tile([128, 1], fp32, tag=f"rank{c}")
                sb_ranks.append(rk)
            kv_iter = iter(kv_copy_jobs)
            kv_per_d = (len(kv_copy_jobs) + RANK_D - 1) // max(RANK_D, 1)
            kv_ct = [0]
            for c in range(NCH):
                e = rank_asgn[c]
                if e == "A":
                    scr = scrAp.tile([128, S], fp32, tag="scrA")
                    rsgn = spool.tile([128, 1], fp32, tag=f"rsgn{c}")
                    nc.scalar.activation(scr[:], sb_bc[:], Act.Sign,
                                         bias=sb_stok[:, c:c + 1],
                                         scale=-1.0, accum_out=rsgn[:])
                    nc.vector.tensor_scalar(sb_ranks[c][:], rsgn[:], -0.5,
                                            1023.5, Alu.mult, Alu.add)
                elif e == "D":
                    scr = scrDp.tile([128, S], fp32, tag="scrD")
                    nc.vector.tensor_scalar(
                        scr[:], sb_bc[:], sb_stok[:, c:c + 1], 0.0,
                        Alu.is_gt, Alu.add, accum_out=sb_ranks[c][:])
                    for _ in range(kv_per_d):
                        job = next(kv_iter, None)
                        if job is not None:
                            nc.vector.tensor_copy(job[0][:], job[1][:])
                else:
                    scr = scrPp.tile([128, S], fp32, tag="scrP")
                    nc.gpsimd.tensor_scalar(
                        scr[:], sb_bc[:], sb_stok[:, c:c + 1], 0.0,
                        Alu.is_gt, Alu.add, accum_out=sb_ranks[c][:])
            for job in kv_iter:
                nc.vector.tensor_copy(job[0][:], job[1][:])
            # qT2 copy rides after ACT's rank chunks
            nc.scalar.copy(sb_qT2[:], ps_q[:])
            es_d.close()

            # ---- stage C2: one-hot gather of the top-K kv rows (bf16) ----
            ps_kvsel_full = pCs.tile([128, 512], fp32)   # 1 bank
            ps_kvsel = ps_kvsel_full[:, 0:KP]
            for c in range(NCH):
                oh = scrPp.tile([128, K], bf16, tag="oh")
                nc.gpsimd.tensor_scalar(oh[:], sb_iota, sb_ranks[c][:], None,
                                        Alu.is_equal)
                kvch = sb_kvtoks[c][:]
                nc.tensor.matmul(ps_kvsel[0:H1, :], kvch, oh[:, 0:KP],
                                 start=(c == 0), stop=False,
                                 skip_group_check=True)
                nc.tensor.matmul(ps_kvsel[H1:128, :], kvch, oh[:, KP:K],
                                 start=(c == 0), stop=(c == NCH - 1),
                                 skip_group_check=True)
            sb_kvb2a = spool.tile([128, 51], fp32)
            nc.vector.tensor_copy(sb_kvb2a[:], ps_kvsel[:, 0:51])
            sb_kvb2b = spool.tile([128, KP - 51], fp32)
            nc.vector.tensor_copy(sb_kvb2b[:], ps_kvsel[:, 51:KP])
            es_c.close()

            def kvb_col(p):
                return (sb_kvb2a[:, p:p + 1] if p < 51
                        else sb_kvb2b[:, p - 51:p - 50])

            pmain = es.enter_context(
                tc.tile_pool(name="main_psum", bufs=3, space="PSUM"))
            pout = es.enter_context(
                tc.tile_pool(name="out_psum", bufs=1, space="PSUM"))

            # ---- stage E: main pairwise loop ----
            # Group g covers tokens 4g..4g+3 (2 slot-pairs).  h2 of two
            # consecutive groups shares one [128, 2048] fp8 supertile whose
            # halves are the DoubleRow k-tiles of the mw3 contraction.
            ps_out = pout.tile([D, SH], fp32, tag="outacc")
            lhsT_dr = sb_pk8[:, 0:32].rearrange("p (k m) -> p k m", k=2)
            sup = None
            pending_w4 = []
            first_w4 = [True]
            g_order = list(range(NG))

            def emit_w4(sup_t, last=False):
                rhs3 = sup_t[:, 0:2 * SH].rearrange("p (k q) -> p k q", k=2)
                for half in range(2):
                    sl = slice(half * 512, (half + 1) * 512)
                    nc.tensor.matmul(ps_out[:, sl], lhsT_dr, rhs3[:, :, sl],
                                     perf_mode=DR, start=first_w4[0],
                                     stop=last and half == 1,
                                     skip_group_check=True)
                first_w4[0] = False

            nreg = [0]
            for gi, g in enumerate(g_order):
                par = nreg[0] % 2
                if g != NG - 1:
                    nreg[0] += 1
                    if par == 0:
                        sup = h2pool.tile([128, 2 * SH], fp8, tag="sup")
                ps_h = pmain.tile([128, SH], fp32, tag="hps")
                for half in range(2):
                    p = 2 * g + half
                    h1t = h1pool.tile([128, SH], bf16, tag="h1")
                    if h1_asgn[p] == "P":
                        nc.gpsimd.tensor_scalar(h1t[:], sb_qT2[:],
                                                kvb_col(p), 0.0,
                                                Alu.add, Alu.max)
                    else:
                        nc.vector.tensor_scalar(h1t[:], sb_qT2[:],
                                                kvb_col(p), 0.0,
                                                Alu.add, Alu.max)
                    rows = slice(half * H1, (half + 1) * H1)
                    for n in range(2):
                        sl = slice(n * 512, (n + 1) * 512)
                        nc.tensor.matmul(ps_h[rows, sl], sb_bdmw2,
                                         h1t[:, sl], start=True, stop=True)
                if g == NG - 1:
                    # odd 51st group: fp8 tail
                    tail = h2pool.tile([128, SH], fp8, tag="tail")
                    h2dst = tail[:]
                    split = False
                else:
                    h2dst = sup[:, par * SH:(par + 1) * SH]
                    split = h2_asgn[g] != "A"
                if not split:
                    nc.scalar.activation(h2dst, ps_h[:], Act.Relu,
                                         bias=sb_mb24)
                else:
                    nc.scalar.activation(h2dst[:, 0:512], ps_h[:, 0:512],
                                         Act.Relu, bias=sb_mb24)
                    nc.vector.tensor_scalar(h2dst[:, 512:SH],
                                            ps_h[:, 512:SH], sb_mb24, 0.0,
                                            Alu.add, Alu.max)
                if g != NG - 1 and par == 1:
                    pending_w4.append(sup)
                    if len(pending_w4) > 1:
                        emit_w4(pending_w4.pop(0))
            for s_t in pending_w4:
                emit_w4(s_t)
            # tail group: two zero-ktile DR matmuls — the rhs k-tiles are the
            # two query-halves of the tail tile; the unused half multiplies
            # zero weights (pk8 cols 32:96 hold [w3|0] and [0|w3])
            tail3 = tail[:, 0:SH].rearrange("p (k q) -> p k q", k=2)
            lhsT_t0 = sb_pk8[:, 32:64].rearrange("p (k m) -> p k m", k=2)
            lhsT_t1 = sb_pk8[:, 64:96].rearrange("p (k m) -> p k m", k=2)
            nc.tensor.matmul(ps_out[:, 0:512], lhsT_t0, tail3,
                             perf_mode=DR, start=False, stop=False,
                             skip_group_check=True)
            nc.tensor.matmul(ps_out[:, 512:SH], lhsT_t1, tail3,
                             perf_mode=DR, start=False, stop=True,
                             skip_group_check=True)

            # ---- stage F: scale + bias + store ----
            sb_out = spool.tile([D, SH], fp32)
            nc.scalar.activation(sb_out[:], ps_out[:], Act.Identity,
                                 bias=sb_mb3f, scale=INV_K)
            nc.sync.dma_start(d_outT[:], sb_out[:])

    nc.compile()
    return nc


def _host_inputs(full, sw1, sb1, sw2, sb2, mw1, mb1, mw2, mb2, mw3, mb3):
    """Build the 8 per-core input maps (host-side sharding + layout prep)."""
    import ml_dtypes
    f32 = np.float32
    bf16 = ml_dtypes.bfloat16
    fp8 = ml_dtypes.float8_e4m3
    full = np.asarray(full, dtype=f32)
    ones_row = np.ones((1, S), dtype=f32)

    pksc = np.zeros((32, 160), dtype=f32)
    pksc[0:DA, 0:32] = np.concatenate(
        [np.asarray(sw1, dtype=f32), np.asarray(sb1, dtype=f32)[None, :]],
        axis=0)
    pksc[0:H2, 32:160] = np.tile(np.asarray(sw2, dtype=f32).reshape(H2, 1),
                                 (1, 128))

    pkf = np.zeros((128, 208), dtype=f32)
    pkf[0, 206] = 1.0
    pkf[:, 0:KP] = np.arange(0, K, 2, dtype=f32)[None, :]
    pkf[:, KP:K] = np.arange(1, K, 2, dtype=f32)[None, :]
    pkf[:, 204] = np.tile(np.asarray(mb2, dtype=f32), 4)
    pkf[:, 205] = np.tile(np.asarray(mb3, dtype=f32), 8)

    pkb = np.zeros((128, 192), dtype=f32)
    pkb[0:H1, 0:H2] = mw2
    pkb[H1:128, H2:H1] = mw2
    pkb[0:D, 64:128] = mw1[:D]
    pkb[0:DA, 128:192] = np.concatenate(
        [np.asarray(mw1[D:2 * D] + mw1[2 * D:], dtype=f32),
         np.asarray(mb1, dtype=f32)[None, :]], axis=0)
    pkb = pkb.astype(bf16)

    # 8 dither-quantized copies of mw3*16: copy (block b, ktile i) at
    # pk8[32b:32b+32, 16i:16i+16]; per weight, copies alternate between
    # the two fp8 neighbours bracketing the true value so that the
    # average over copies tracks it to ~1/16 ulp
    W = np.asarray(mw3, dtype=f32) * MW3_SCALE            # [32, 16]
    fp8_vals = np.sort(np.unique(
        np.arange(256, dtype=np.uint8).view(fp8).astype(f32)))
    fp8_vals = fp8_vals[np.isfinite(fp8_vals)]
    lo_idx = np.searchsorted(fp8_vals, W, side="right") - 1
    lo = fp8_vals[np.clip(lo_idx, 0, len(fp8_vals) - 1)]
    hi = fp8_vals[np.clip(lo_idx + 1, 0, len(fp8_vals) - 1)]
    denom = np.where(hi > lo, hi - lo, 1.0)
    n_hi = np.round((W - lo) / denom * 8.0)
    pk8 = np.zeros((128, 96), dtype=f32)
    for b in range(4):
        for i in range(2):
            c = 2 * b + i
            cpy = np.where(c < n_hi, hi, lo)
            pk8[32 * b:32 * b + 32, 16 * i:16 * i + 16] = cpy
            # tail lhsTs: [w3|0] at cols 32:64, [0|w3] at 64:96
            pk8[32 * b:32 * b + 32, 32 + 16 * i:48 + 16 * i] = (
                cpy if i == 0 else 0.0)
            pk8[32 * b:32 * b + 32, 64 + 16 * i:80 + 16 * i] = (
                0.0 if i == 0 else cpy)
    pk8 = pk8.astype(fp8)

    shared = dict(pksc=pksc, pkf=pkf, pkb=pkb, pk8=pk8)
    in_maps = []
    for c in range(N_CORES):
        b, h = c // 2, c % 2
        fbT = np.concatenate(
            [np.ascontiguousarray(full[b].T), ones_row], axis=0)
        m = dict(shared)
        m["fbT"] = fbT
        m["fbTb"] = fbT.astype(bf16)
        m["fqT"] = np.ascontiguousarray(
            full[b, h * SH:(h + 1) * SH, :].T).astype(bf16)
        in_maps.append(m)
    return in_maps


def get_module():
    if "nc" not in _cache:
        _cache["nc"] = _build_module()
    return _cache["nc"]


def run_cores(in_maps):
    from concourse.bass_utils import run_bass_kernel_spmd
    nc = get_module()
    return run_bass_kernel_spmd(nc, in_maps, list(range(N_CORES))).results


def kernel(full, sw1, sb1, sw2, sb2, mw1, mb1, mw2, mb2, mw3, mb3):
    in_maps = _host_inputs(full, sw1, sb1, sw2, sb2, mw1, mb1, mw2, mb2,
                           mw3, mb3)
    results = run_cores(in_maps)
    out = np.empty((B, S, D), dtype=np.float32)
    for c in range(N_CORES):
        b, h = c // 2, c % 2
        out[b, h * SH:(h + 1) * SH, :] = results[c]["outT"].T
    return out
